# revision 1
# baseline (speedup 1.0000x reference)
"""PlatonicConv (graph-mode attention) Trainium2 Bass kernel.

Math (per graph of 64 fully-connected nodes, 24 group-heads of dim 16):
  q/k/v = x @ W; RoPE(q, k) from pos; S = q.k^T/4; softmax over dst;
  out = A @ v; y = out @ Wo.  32 graphs -> data-parallel over 8 cores.

Key layout choices (per core: 4 graphs, 256 nodes):
  * Everything attention-side lives transposed ([feature, node]) so the
    per-head score matmuls need no activation transposes.
  * Heads are "spread" to 32-aligned partition slots so score matmuls
    pack 4-way into the PE array via tile_position row groups.
  * Softmax is max-free (scores are O(1) by construction); the denominator
    comes for free as a 17th row of each AV matmul via an interleaved
    ones-column in the V weights.
"""

import numpy as np

G = 12
H = 2
D = 16
GH = 24          # G * H group-heads
C = 384          # in/emb/out channels
NG = 32          # graphs
NPG = 64         # nodes per graph
N = NG * NPG
NCORES = 8
GPC = NG // NCORES   # graphs per core = 4
NPC = GPC * NPG      # nodes per core = 256
VW = 17              # V block width (16 + ones col)
CAUG = GH * VW       # 408

_F32R_PROJ = True    # big projections in float32r (4x faster PE)

_CACHE = {}


def _host_prep(Wq, Wk, Wv, Wo, rope_freqs):
    f32 = np.float32
    idx = np.arange(C)
    d16 = idx % 16
    partner = np.where(d16 % 2 == 0, idx + 1, idx - 1)
    sign = np.where(d16 % 2 == 0, -1.0, 1.0).astype(f32)
    # pair-swap-negated projections: QpT = Wqp^T X^T, Qp^T[e] = sign(e) Q^T[partner(e)]
    Wqp = (Wq[:, partner] * sign[None, :]).astype(f32)
    Wkp = (Wk[:, partner] * sign[None, :]).astype(f32)

    # V interleaved with a ones column per head: block j = [Wv head j | 0]
    Wvil = np.zeros((C, CAUG), f32)
    for j in range(GH):
        Wvil[:, VW * j:VW * j + 16] = Wv[:, 16 * j:16 * j + 16]
    vseed = np.zeros((1, CAUG), f32)
    vseed[0, VW * np.arange(GH) + 16] = 1.0

    # rope freq pattern [3, 64] for the COMPACT layout: row r = 16m + d of a
    # 64-row block belongs to head-in-block m -> h = m%2, pair w = d//2
    fr = rope_freqs.astype(f32)            # [3, 2, 8]
    fpat = np.zeros((3, 64), f32)
    for r in range(64):
        fpat[:, r] = fr[:, (r // 16) % 2, (r % 16) // 2]

    # spread matrix: compact rows (16/head) -> 32-aligned slots; two stacked
    # copies so odd 64-row slabs can use base partition 64
    esp2 = np.zeros((128, 128), f32)
    for k in range(64):
        m = 32 * (k // 16) + (k % 16)
        esp2[k, m] = 1.0
        esp2[64 + k, m] = 1.0

    # normalization broadcast: row j of rden -> 16 consecutive emb rows
    e24 = np.zeros((GH, C), f32)
    e24[idx // 16, idx] = 1.0

    onesrow = np.ones((1, 128), f32)
    ident = np.eye(128, dtype=f32)
    def pack(w):
        # [384, cols] -> [128, 3*cols]: row p = concat_s w[128 s + p]
        cols = w.shape[1]
        return np.ascontiguousarray(
            w.reshape(3, 128, cols).transpose(1, 0, 2).reshape(128, 3 * cols)
            .astype(f32))

    return dict(
        wq=pack(Wq), wqp=pack(Wqp), wk=pack(Wk), wkp=pack(Wkp),
        wvil=pack(Wvil), wo=pack(Wo),
        vseed=vseed, fpat=fpat, esp2=esp2, e24=e24,
        onesrow=onesrow, ident=ident,
    )


def _build_nc():
    import concourse.bacc as bacc
    import concourse.tile as tile
    import concourse.mybir as mybir
    from contextlib import ExitStack

    f32 = mybir.dt.float32
    fmm = mybir.dt.float32r if _F32R_PROJ else f32
    AF = mybir.ActivationFunctionType

    nc = bacc.Bacc("TRN2", target_bir_lowering=False)

    x_d = nc.dram_tensor("x", [128, 2 * C], f32, kind="ExternalInput")
    posT_d = nc.dram_tensor("posT", [3, NPC], f32, kind="ExternalInput")
    wq_d = nc.dram_tensor("wq", [128, 3 * C], fmm, kind="ExternalInput")
    wqp_d = nc.dram_tensor("wqp", [128, 3 * C], fmm, kind="ExternalInput")
    wk_d = nc.dram_tensor("wk", [128, 3 * C], fmm, kind="ExternalInput")
    wkp_d = nc.dram_tensor("wkp", [128, 3 * C], fmm, kind="ExternalInput")
    wvil_d = nc.dram_tensor("wvil", [128, 3 * CAUG], fmm, kind="ExternalInput")
    wo_d = nc.dram_tensor("wo", [128, 3 * C], fmm, kind="ExternalInput")
    vseed_d = nc.dram_tensor("vseed", [1, CAUG], fmm, kind="ExternalInput")
    fpat_d = nc.dram_tensor("fpat", [3, 64], f32, kind="ExternalInput")
    esp2_d = nc.dram_tensor("esp2", [128, 128], fmm, kind="ExternalInput")
    e24_d = nc.dram_tensor("e24", [GH, C], f32, kind="ExternalInput")
    ones_d = nc.dram_tensor("onesrow", [1, 128], fmm, kind="ExternalInput")
    ident_d = nc.dram_tensor("ident", [128, 128], f32, kind="ExternalInput")
    y_d = nc.dram_tensor("y", [NPC, C], f32, kind="ExternalOutput")

    ctx = ExitStack()
    with tile.TileContext(nc) as tc, ctx:
        consts = ctx.enter_context(tc.tile_pool(name="consts", bufs=1))
        wpool = ctx.enter_context(tc.tile_pool(name="weights", bufs=1))
        sb = ctx.enter_context(tc.tile_pool(name="sbuf", bufs=1))
        # general psum: shared tag -> 4 recycled 1-bank slots
        ps_gp = ctx.enter_context(tc.tile_pool(name="ps_gp", bufs=2, space="PSUM"))
        ps_att = ctx.enter_context(tc.tile_pool(name="ps_att", bufs=1, space="PSUM"))
        ps_av = ctx.enter_context(tc.tile_pool(name="ps_av", bufs=2, space="PSUM"))

        def gpt(shape):
            return ps_gp.tile(shape, f32, tag="pp", name="pp")

        # ---- inputs first (x feeds the transposes while weights stream),
        # weights split across the two HWDGE queues (sync + scalar) ----
        xsb = sb.tile([128, 2, C], f32, tag="x")
        nc.sync.dma_start(out=xsb, in_=x_d.rearrange("p (s e) -> p s e", s=2))
        ident = consts.tile([128, 128], f32, tag="ident")
        nc.scalar.dma_start(out=ident, in_=ident_d[:])
        posT = consts.tile([3, NPC], f32, tag="posT")
        nc.scalar.dma_start(out=posT, in_=posT_d[:])
        fpat = consts.tile([3, 64], f32, tag="fpat")
        nc.scalar.dma_start(out=fpat, in_=fpat_d[:])
        esp2 = consts.tile([128, 128], fmm, tag="esp2")
        nc.scalar.dma_start(out=esp2, in_=esp2_d[:])
        e24 = consts.tile([GH, C], f32, tag="e24")
        nc.scalar.dma_start(out=e24, in_=e24_d[:])
        vseed = consts.tile([1, CAUG], fmm, tag="vseed")
        nc.scalar.dma_start(out=vseed, in_=vseed_d[:])
        onesrow = consts.tile([1, 128], fmm, tag="ones")
        nc.scalar.dma_start(out=onesrow, in_=ones_d[:])

        def load_w(dram, cols, tag, eng):
            t = wpool.tile([128, 3, cols], fmm, tag=tag)
            dv = dram.rearrange("p (s e) -> p s e", s=3)
            for s in range(3):
                eng.dma_start(out=t[:, s, :], in_=dv[:, s, :])
            return t
        wq = load_w(wq_d, C, "wq", nc.sync)
        wqp = load_w(wqp_d, C, "wqp", nc.scalar)
        wk = load_w(wk_d, C, "wk", nc.sync)
        wkp = load_w(wkp_d, C, "wkp", nc.scalar)
        wvil = load_w(wvil_d, CAUG, "wvil", nc.sync)
        wo = load_w(wo_d, C, "wo", nc.scalar)

        # ---- X^T [384, 256] via PE transposes ----
        xT = []
        for j in range(3):
            t = sb.tile([128, NPC], fmm, tag=f"xT{j}")
            for i in range(2):
                pst = gpt([128, 128])
                nc.tensor.transpose(
                    out=pst, in_=xsb[:, i, 128 * j:128 * j + 128], identity=ident)
                nc.vector.tensor_copy(out=t[:, 128 * i:128 * i + 128], in_=pst)
            xT.append(t)

        # ---- theta pattern + cos/sin [64, 256] -> stacked [128, 256] ----
        thps = gpt([64, NPC])
        nc.tensor.matmul(out=thps, lhsT=fpat, rhs=posT, start=True, stop=True)
        # range-reduce into [-pi, pi] for the scalar-engine Sin table:
        #   tr = t - 2pi*rint(t/2pi)  (f32<->i32 convert rounds to nearest)
        PI = float(np.pi)
        thc = sb.tile([64, NPC], f32, tag="thc")
        nc.vector.tensor_scalar_add(thc, thps, PI / 2)   # cos(t) = sin(t + pi/2)

        def range_reduce(src, tag):
            # robust to int-convert rounding mode (trunc on sim, rint on hw):
            # u = t+16pi > 0; v = u - 2pi*cvt(u/2pi) in [-pi, 2pi); then
            # subtract 2pi where v > pi  ->  [-pi, pi]
            u = sb.tile([64, NPC], f32, tag=f"u{tag}", name="u")
            nc.vector.tensor_scalar_add(u, src, 16 * PI)
            m1 = sb.tile([64, NPC], f32, tag=f"m1{tag}", name="m1")
            nc.vector.tensor_scalar_mul(m1, u, 1.0 / (2 * PI))
            ni = sb.tile([64, NPC], mybir.dt.int32, tag=f"ni{tag}", name="ni")
            nc.vector.tensor_copy(out=ni, in_=m1)
            nf = sb.tile([64, NPC], f32, tag=f"nf{tag}", name="nf")
            nc.vector.tensor_copy(out=nf, in_=ni)
            v = sb.tile([64, NPC], f32, tag=f"v{tag}", name="v")
            nc.vector.scalar_tensor_tensor(
                out=v, in0=nf, scalar=-2 * PI, in1=u,
                op0=mybir.AluOpType.mult, op1=mybir.AluOpType.add)
            mk = sb.tile([64, NPC], f32, tag=f"mk{tag}", name="mk")
            nc.vector.tensor_scalar(out=mk, in0=v, scalar1=PI, scalar2=None,
                                    op0=mybir.AluOpType.is_gt)
            red = sb.tile([64, NPC], f32, tag=f"red{tag}", name="red")
            nc.vector.scalar_tensor_tensor(
                out=red, in0=mk, scalar=-2 * PI, in1=v,
                op0=mybir.AluOpType.mult, op1=mybir.AluOpType.add)
            return red

        thr_s = range_reduce(thps, "s")
        thr_c = range_reduce(thc, "c")
        cpat = sb.tile([64, NPC], f32, tag="cpat")
        nc.scalar.activation(out=cpat, in_=thr_c, func=AF.Sin)
        spat = sb.tile([64, NPC], f32, tag="spat")
        nc.scalar.activation(out=spat, in_=thr_s, func=AF.Sin)
        cosf = sb.tile([128, NPC], f32, tag="cosf")
        sinf = sb.tile([128, NPC], f32, tag="sinf")
        for half in range(2):
            nc.sync.dma_start(out=cosf[64 * half:64 * half + 64, :], in_=cpat)
            nc.sync.dma_start(out=sinf[64 * half:64 * half + 64, :], in_=spat)

        # ---- projections (transposed) + compact RoPE + spread, per m-slab ----
        def proj_m(w, m):
            ps = gpt([128, NPC])
            for k in range(3):
                nc.tensor.matmul(
                    out=ps,
                    lhsT=w[:, k, 128 * m:128 * m + 128],
                    rhs=xT[k],
                    start=(k == 0), stop=(k == 2))
            return ps

        def rope_spread(w, wp, tag):
            """rotated + spread [768, 256] as 6 sbuf tiles."""
            spread = []
            for m in range(3):
                qt = proj_m(w, m)
                qpt = proj_m(wp, m)
                a = sb.tile([128, NPC], f32, tag=f"ra{tag}{m}")
                b = sb.tile([128, NPC], f32, tag=f"rb{tag}{m}")
                nc.vector.tensor_mul(out=a, in0=qt, in1=cosf)
                nc.vector.tensor_mul(out=b, in0=qpt, in1=sinf)
                rot = sb.tile([128, NPC], fmm, tag=f"rot{tag}{m}")
                nc.vector.tensor_add(out=rot, in0=a, in1=b)
                for half in range(2):
                    sp = gpt([128, NPC])
                    nc.tensor.matmul(
                        out=sp,
                        lhsT=esp2[64 * half:64 * half + 64, :],
                        rhs=rot[64 * half:64 * half + 64, :],
                        start=True, stop=True)
                    t = sb.tile([128, NPC], f32, tag=f"sps{tag}{2 * m + half}")
                    nc.vector.tensor_copy(out=t, in_=sp)
                    spread.append(t)
            return spread

        qsp = rope_spread(wq, wqp, "q")
        ksp = rope_spread(wk, wkp, "k")

        # ---- V_aug [256, 408] untransposed (+ ones cols via K=1 matmul) ----
        vau = []
        for i in range(2):
            ps = gpt([128, CAUG])
            for k in range(3):
                nc.tensor.matmul(
                    out=ps,
                    lhsT=xT[k][:, 128 * i:128 * i + 128],
                    rhs=wvil[:, k, :],
                    start=(k == 0), stop=False)
            nc.tensor.matmul(
                out=ps, lhsT=onesrow, rhs=vseed,
                start=False, stop=True)
            t = sb.tile([128, CAUG], f32, tag=f"vau{i}")
            nc.vector.tensor_copy(out=t, in_=ps)
            vau.append(t)

        # ---- scores S^T + exp, per graph-pair.
        # Concurrent row-tiled matmuls MUST land in distinct PSUM banks:
        # head gh -> bank gh%4 (512-col block), col 64*(gh//4), rows 64*g01.
        def scol(gh):
            return 512 * (gh % 4) + 64 * (gh // 4)

        expst = []
        for pair in range(2):
            stp = ps_att.tile([128, 4 * 512], f32, tag="stps")
            for gh in range(GH):
                tilei, slot = divmod(gh, 4)
                lo = 32 * slot
                for g01 in range(2):
                    g = 2 * pair + g01
                    nc.tensor.matmul(
                        out=stp[64 * g01:64 * g01 + 64, scol(gh):scol(gh) + 64],
                        lhsT=ksp[tilei][lo:lo + 16, 64 * g:64 * g + 64],
                        rhs=qsp[tilei][lo:lo + 16, 64 * g:64 * g + 64],
                        start=True, stop=True,
                        tile_position=(lo, 64 * g01))
            et = sb.tile([128, 4 * 512], f32, tag=f"expst{pair}")
            for b in range(4):
                nc.scalar.activation(
                    out=et[:, 512 * b:512 * b + 384],
                    in_=stp[:, 512 * b:512 * b + 384],
                    func=AF.Exp, scale=0.25)
            expst.append(et)

        # ---- AV (+den row): per (quad, parity) [128, 128] psum tiles so the
        # two concurrent row groups (graph parities) use distinct banks;
        # head slot 32*(gh%4) rows, col 64*(g//2) ----
        avsb = sb.tile([128, 6 * 256], f32, tag="avsb")
        dens = sb.tile([GH, NPC], f32, tag="dens")
        og = [sb.tile([128, NPC], f32, tag=f"og{m}", name="og") for m in range(3)]
        for qd in range(6):
            avt = [ps_av.tile([128, 128], f32, tag="av", name="av")
                   for _ in range(2)]
            nc.vector.memset(avt[0], 0.0)
            nc.vector.memset(avt[1], 0.0)
            for a in range(4):
                gh = 4 * qd + a
                for g in range(GPC):
                    pair, g01 = divmod(g, 2)
                    lo = 64 * g01
                    nc.tensor.matmul(
                        out=avt[g01][32 * a:32 * a + VW,
                                     64 * (g // 2):64 * (g // 2) + 64],
                        lhsT=vau[pair][lo:lo + 64, VW * gh:VW * gh + VW],
                        rhs=expst[pair][lo:lo + 64, scol(gh):scol(gh) + 64],
                        start=True, stop=True,
                        tile_position=(lo, 32 * a))
            for g01 in range(2):
                nc.vector.tensor_copy(
                    out=avsb[:, 256 * qd + 128 * g01:256 * qd + 128 * g01 + 128],
                    in_=avt[g01])
            # per-quad gathers overlap the remaining AV quads; columns stay in
            # avsb's (g01, pair, i) node order (final DRAM write undoes it)
            cs = slice(256 * qd, 256 * qd + 256)
            for a in range(4):
                gh = 4 * qd + a
                dst, row = divmod(16 * gh, 128)
                nc.scalar.dma_start(
                    out=dens[gh:gh + 1, :],
                    in_=avsb[32 * a + 16:32 * a + 17, cs])
                (nc.sync if a % 2 else nc.scalar).dma_start(
                    out=og[dst][row:row + 16, :],
                    in_=avsb[32 * a:32 * a + 16, cs])
        rden = sb.tile([GH, NPC], f32, tag="rden")
        nc.vector.reciprocal(out=rden, in_=dens)
        onrm = []
        for m in range(3):
            rt = gpt([128, NPC])
            nc.tensor.matmul(
                out=rt, lhsT=e24[:, 128 * m:128 * m + 128],
                rhs=rden, start=True, stop=True)
            t = sb.tile([128, NPC], fmm, tag=f"onrm{m}")
            nc.vector.tensor_mul(out=t, in0=og[m], in1=rt)
            onrm.append(t)

        # ---- y = O_norm @ Wo ----
        for i in range(2):
            yps = gpt([128, C])
            for k in range(3):
                nc.tensor.matmul(
                    out=yps,
                    lhsT=onrm[k][:, 128 * i:128 * i + 128],
                    rhs=wo[:, k, :],
                    start=(k == 0), stop=(k == 2))
            ysb = sb.tile([128, C], f32, tag=f"ysb{i}", name="ysb")
            nc.vector.tensor_copy(out=ysb, in_=yps)
            # node column order downstream of avsb is (g01, pair, i); psum
            # y-tile i covers g01 == i, rows (pair, i64) -> node 128*pair + 64*i + i64
            nc.sync.dma_start(
                out=y_d.rearrange("(pr b i) e -> b pr i e", pr=2, b=2, i=64)[i],
                in_=ysb)

    nc.compile()
    return nc


def _get_nc():
    if "nc" not in _CACHE:
        _CACHE["nc"] = _build_nc()
    return _CACHE["nc"]


def make_in_maps(inputs):
    x = np.asarray(inputs["x"], np.float32)
    pos = np.asarray(inputs["pos"], np.float32)
    prep = _host_prep(np.asarray(inputs["Wq"], np.float32),
                      np.asarray(inputs["Wk"], np.float32),
                      np.asarray(inputs["Wv"], np.float32),
                      np.asarray(inputs["Wo"], np.float32),
                      np.asarray(inputs["rope_freqs"], np.float32))
    in_maps = []
    for c in range(NCORES):
        sl = slice(c * NPC, (c + 1) * NPC)
        m = dict(prep)
        xs = x[sl]
        m["x"] = np.ascontiguousarray(
            xs.reshape(2, 128, C).transpose(1, 0, 2).reshape(128, 2 * C))
        m["posT"] = np.ascontiguousarray(pos[sl].T)
        in_maps.append(m)
    return in_maps


def kernel(**inputs):
    from concourse.bass_utils import run_bass_kernel_spmd

    in_maps = make_in_maps(inputs)

    nc = _get_nc()
    res = run_bass_kernel_spmd(nc, in_maps, core_ids=list(range(NCORES)))
    out = np.concatenate([res.results[c]["y"] for c in range(NCORES)], axis=0)
    return out.astype(np.float32)



# revision 6
# speedup vs baseline: 1.3121x; 1.3121x over previous
"""PlatonicConv (graph-mode attention) Trainium2 Bass kernel.

Math (per graph of 64 fully-connected nodes, 24 group-heads of dim 16):
  q/k/v = x @ W; RoPE(q, k) from pos; S = q.k^T/4; softmax over dst;
  out = A @ v; y = out @ Wo.  32 graphs -> data-parallel over 8 cores.

v2 design (per core: 4 graphs, 256 nodes), all matmul operands bf16
(full PE rate at any free dim; fp32/f32r are 4x slower below 256 cols):
  * x^T via PE transposes; q/k projected transposed ([feature, node]).
  * RoPE pair-swap is folded into a second spread matrix (esp2p), so
    rot_spread = esp2 @ (q*cos) + esp2p @ (q*sin) accumulates in PSUM --
    no Wqp/Wkp weights, no separate rotate step.
  * theta and theta+pi/2 come from ONE matmul ([128,512]) via a ones row
    in posTx; one range-reduce + one Sin activation yields sin|cos.
  * Softmax is max-free (scores are O(1) by construction); denominators
    are a 17th row of each AV matmul via an interleaved ones-column in V.
  * AV output stays in "spread" 32-row-slot layout; Wo's rows are
    pre-spread on the host (wo_sp) so no reorganization DMAs are needed.
    Denominators are extracted/broadcast with tiny PE matmuls.
  * Final y lands in natural node order -> one dense DRAM write per
    128-node half, no host-side reorder.
"""

import numpy as np
import ml_dtypes

G = 12
H = 2
D = 16
GH = 24          # G * H group-heads
C = 384          # in/emb/out channels
NG = 32          # graphs
NPG = 64         # nodes per graph
N = NG * NPG
NCORES = 8
GPC = NG // NCORES   # graphs per core = 4
NPC = GPC * NPG      # nodes per core = 256
VW = 17              # V block width (16 + ones col)
CAUG = GH * VW       # 408

BF = ml_dtypes.bfloat16

_CACHE = {}


def _host_prep(Wq, Wk, Wv, Wo, rope_freqs):
    f32 = np.float32

    def pack(w):
        # [384, cols] -> [128, 3*cols]: row p = concat_s w[128 s + p]
        cols = w.shape[1]
        return np.ascontiguousarray(
            w.reshape(3, 128, cols).transpose(1, 0, 2).reshape(128, 3 * cols)
            .astype(BF))

    # V interleaved with a ones column per head: block j = [Wv head j | 0]
    Wvil = np.zeros((C, CAUG), f32)
    for j in range(GH):
        Wvil[:, VW * j:VW * j + 16] = Wv[:, 16 * j:16 * j + 16]
    vseed = np.zeros((1, CAUG), f32)
    vseed[0, VW * np.arange(GH) + 16] = 1.0

    # theta pattern [4, 128] for COMPACT layout rows (dup to both halves):
    # row r=16m+d of a 64-block -> head h=m%2, pair w=d//2. Row 3 = pi/2
    # (multiplies posTx's ones row; only the cos half gets the bias).
    fr = rope_freqs.astype(f32)            # [3, 2, 8]
    fpat3 = np.zeros((4, 128), f32)
    for r in range(128):
        rr = r % 64
        fpat3[:3, r] = fr[:, (rr // 16) % 2, (rr % 16) // 2]
    fpat3[3, :] = np.pi / 2

    # spread matrices: compact rows (16/head) -> 32-aligned slots; two
    # stacked copies so odd 64-row slabs use base partition 64.
    # esp2p folds the RoPE pair-swap: spread(rot2) = esp2p.T @ (q*sin).
    esp2 = np.zeros((128, 128), f32)
    esp2p = np.zeros((128, 128), f32)
    for k in range(64):
        m = 32 * (k // 16) + (k % 16)
        esp2[k, m] = 1.0
        esp2[64 + k, m] = 1.0
        d = k % 16
        mp = m + (1 if d % 2 == 0 else -1)
        sg = 1.0 if d % 2 == 0 else -1.0
        esp2p[k, mp] = sg
        esp2p[64 + k, mp] = sg

    # den extract: per quad qd, spread row 32a+16 -> out row 4qd+a of a
    # [24,128] PSUM accumulation (matmul out base partition must be
    # 32-aligned, so each quad writes the full 24 rows, 4 nonzero)
    esel = np.zeros((128, 6 * GH), f32)
    for qd in range(6):
        for a in range(4):
            esel[32 * a + 16, GH * qd + 4 * qd + a] = 1.0

    # den broadcast: rden row gh -> 16 spread rows of its quad block
    bsel = np.zeros((GH, 6 * 128), f32)
    for gh in range(GH):
        qd, a = divmod(gh, 4)
        bsel[gh, 128 * qd + 32 * a:128 * qd + 32 * a + 16] = 1.0

    # Wo with rows pre-spread to the AV output layout (den/pad rows = 0)
    wo_sp = np.zeros((128, 6 * C), f32)
    for gh in range(GH):
        qd, a = divmod(gh, 4)
        wo_sp[32 * a:32 * a + 16, C * qd:C * qd + C] = Wo[16 * gh:16 * gh + 16]

    return dict(
        wq=pack(Wq), wk=pack(Wk), wvil=pack(Wvil),
        wo_sp=wo_sp.astype(BF), vseed=vseed.astype(BF),
        fpat3=fpat3, esp2=esp2.astype(BF), esp2p=esp2p.astype(BF),
        esel=esel, bsel=bsel.astype(BF),
        onesrow=np.ones((1, 128), BF), ident=np.eye(128, dtype=BF),
    )


def _build_nc():
    import concourse.bacc as bacc
    import concourse.tile as tile
    import concourse.mybir as mybir
    from contextlib import ExitStack

    f32 = mybir.dt.float32
    bf16 = mybir.dt.bfloat16
    AF = mybir.ActivationFunctionType

    nc = bacc.Bacc("TRN2", target_bir_lowering=False)

    x_d = nc.dram_tensor("x", [128, 2 * C], bf16, kind="ExternalInput")
    posTx_d = nc.dram_tensor("posTx", [4, 2 * NPC], f32, kind="ExternalInput")
    wq_d = nc.dram_tensor("wq", [128, 3 * C], bf16, kind="ExternalInput")
    wk_d = nc.dram_tensor("wk", [128, 3 * C], bf16, kind="ExternalInput")
    wvil_d = nc.dram_tensor("wvil", [128, 3 * CAUG], bf16, kind="ExternalInput")
    wo_sp_d = nc.dram_tensor("wo_sp", [128, 6 * C], bf16, kind="ExternalInput")
    vseed_d = nc.dram_tensor("vseed", [1, CAUG], bf16, kind="ExternalInput")
    fpat3_d = nc.dram_tensor("fpat3", [4, 128], f32, kind="ExternalInput")
    esp2_d = nc.dram_tensor("esp2", [128, 128], bf16, kind="ExternalInput")
    esp2p_d = nc.dram_tensor("esp2p", [128, 128], bf16, kind="ExternalInput")
    esel_d = nc.dram_tensor("esel", [128, 6 * GH], f32, kind="ExternalInput")
    bsel_d = nc.dram_tensor("bsel", [GH, 6 * 128], bf16, kind="ExternalInput")
    ones_d = nc.dram_tensor("onesrow", [1, 128], bf16, kind="ExternalInput")
    ident_d = nc.dram_tensor("ident", [128, 128], bf16, kind="ExternalInput")
    y_d = nc.dram_tensor("y", [NPC, C], f32, kind="ExternalOutput")

    ctx = ExitStack()
    with tile.TileContext(nc) as tc, ctx:
        consts = ctx.enter_context(tc.tile_pool(name="consts", bufs=1))
        wpool = ctx.enter_context(tc.tile_pool(name="weights", bufs=1))
        sb = ctx.enter_context(tc.tile_pool(name="sbuf", bufs=1))
        # 2+2+4 PSUM banks: gp (proj/theta/vau/den/bcast/y), sp (spread/AV),
        # att (score tiles; bank gh%4 for 4-way concurrent row groups)
        ps_gp = ctx.enter_context(tc.tile_pool(name="ps_gp", bufs=2, space="PSUM"))
        ps_sp = ctx.enter_context(tc.tile_pool(name="ps_sp", bufs=2, space="PSUM"))
        ps_att = ctx.enter_context(tc.tile_pool(name="ps_att", bufs=1, space="PSUM"))

        def gpt(shape, dt=f32):
            return ps_gp.tile(shape, dt, tag="pp", name="pp")

        def spt(shape, dt=f32):
            return ps_sp.tile(shape, dt, tag="sp", name="sp")

        # ---- inputs first; weights split across the two HWDGE queues ----
        xsb = sb.tile([128, 2, C], bf16, tag="x")
        nc.sync.dma_start(out=xsb, in_=x_d.rearrange("p (s e) -> p s e", s=2))
        ident = consts.tile([128, 128], bf16, tag="ident")
        nc.scalar.dma_start(out=ident, in_=ident_d[:])
        posTx = consts.tile([4, 2 * NPC], f32, tag="posTx")
        nc.scalar.dma_start(out=posTx, in_=posTx_d[:])
        fpat3 = consts.tile([4, 128], f32, tag="fpat3")
        nc.scalar.dma_start(out=fpat3, in_=fpat3_d[:])
        esp2 = consts.tile([128, 128], bf16, tag="esp2")
        nc.scalar.dma_start(out=esp2, in_=esp2_d[:])
        esp2p = consts.tile([128, 128], bf16, tag="esp2p")
        nc.scalar.dma_start(out=esp2p, in_=esp2p_d[:])
        esel = consts.tile([128, 6 * GH], f32, tag="esel")
        nc.scalar.dma_start(out=esel, in_=esel_d[:])
        bsel = consts.tile([GH, 6 * 128], bf16, tag="bsel")
        nc.scalar.dma_start(out=bsel, in_=bsel_d[:])
        vseed = consts.tile([1, CAUG], bf16, tag="vseed")
        nc.scalar.dma_start(out=vseed, in_=vseed_d[:])
        onesrow = consts.tile([1, 128], bf16, tag="ones")
        nc.scalar.dma_start(out=onesrow, in_=ones_d[:])

        wq = wpool.tile([128, 3, C], bf16, tag="wq")
        nc.sync.dma_start(out=wq, in_=wq_d.rearrange("p (s e) -> p s e", s=3))
        wvil = wpool.tile([128, 3, CAUG], bf16, tag="wvil")
        nc.sync.dma_start(out=wvil, in_=wvil_d.rearrange("p (s e) -> p s e", s=3))
        wk = wpool.tile([128, 3, C], bf16, tag="wk")
        nc.scalar.dma_start(out=wk, in_=wk_d.rearrange("p (s e) -> p s e", s=3))
        wo_sp = wpool.tile([128, 6 * C], bf16, tag="wo_sp")
        nc.scalar.dma_start(out=wo_sp, in_=wo_sp_d[:])

        # ---- X^T [384, 256] via PE transposes (bf16) ----
        xT = []
        for j in range(3):
            t = sb.tile([128, NPC], bf16, tag=f"xT{j}")
            for i in range(2):
                pst = gpt([128, 128], bf16)
                nc.tensor.transpose(
                    out=pst, in_=xsb[:, i, 128 * j:128 * j + 128], identity=ident)
                nc.vector.tensor_copy(out=t[:, 128 * i:128 * i + 128], in_=pst)
            xT.append(t)

        # ---- theta|theta+pi/2 [128, 512] in one matmul; range-reduce into
        # [-pi, pi] for the scalar Sin table; one Sin -> sin|cos pattern ----
        thps = gpt([128, 2 * NPC])
        nc.tensor.matmul(out=thps, lhsT=fpat3, rhs=posTx, start=True, stop=True)
        PI = float(np.pi)
        W2 = 2 * NPC
        # u = t+16pi > 0; v = u - 2pi*cvt(u/2pi) in [-pi, 2pi); subtract 2pi
        # where v > pi (robust to trunc-vs-rint int conversion)
        u = sb.tile([128, W2], f32, tag="rr_u")
        nc.vector.tensor_scalar_add(u, thps, 16 * PI)
        m1 = sb.tile([128, W2], f32, tag="rr_m1")
        nc.vector.tensor_scalar_mul(m1, u, 1.0 / (2 * PI))
        ni = sb.tile([128, W2], mybir.dt.int32, tag="rr_ni")
        nc.vector.tensor_copy(out=ni, in_=m1)
        nf = sb.tile([128, W2], f32, tag="rr_nf")
        nc.vector.tensor_copy(out=nf, in_=ni)
        v = sb.tile([128, W2], f32, tag="rr_v")
        nc.vector.scalar_tensor_tensor(
            out=v, in0=nf, scalar=-2 * PI, in1=u,
            op0=mybir.AluOpType.mult, op1=mybir.AluOpType.add)
        mk = sb.tile([128, W2], f32, tag="rr_mk")
        nc.vector.tensor_scalar(out=mk, in0=v, scalar1=PI, scalar2=None,
                                op0=mybir.AluOpType.is_gt)
        thr = sb.tile([128, W2], f32, tag="rr_red")
        nc.vector.scalar_tensor_tensor(
            out=thr, in0=mk, scalar=-2 * PI, in1=v,
            op0=mybir.AluOpType.mult, op1=mybir.AluOpType.add)
        cs = sb.tile([128, W2], f32, tag="cs")
        nc.scalar.activation(out=cs, in_=thr, func=AF.Sin)
        spat = cs[:, 0:NPC]
        cpat = cs[:, NPC:2 * NPC]

        # ---- projections (transposed) + RoPE + spread, per m-slab ----
        def proj_m(w, m):
            ps = gpt([128, NPC])
            for k in range(3):
                nc.tensor.matmul(
                    out=ps,
                    lhsT=w[:, k, 128 * m:128 * m + 128],
                    rhs=xT[k],
                    start=(k == 0), stop=(k == 2))
            return ps

        def rope_spread(w, tag):
            """rotated + spread [768, 256] as 6 bf16 sbuf tiles."""
            spread = []
            for m in range(3):
                qt = proj_m(w, m)
                a = sb.tile([128, NPC], bf16, tag=f"ra{tag}{m}")
                b = sb.tile([128, NPC], bf16, tag=f"rb{tag}{m}")
                nc.vector.tensor_mul(out=a, in0=qt, in1=cpat)
                nc.vector.tensor_mul(out=b, in0=qt, in1=spat)
                for half in range(2):
                    sp = spt([128, NPC])
                    nc.tensor.matmul(
                        out=sp,
                        lhsT=esp2[64 * half:64 * half + 64, :],
                        rhs=a[64 * half:64 * half + 64, :],
                        start=True, stop=False)
                    nc.tensor.matmul(
                        out=sp,
                        lhsT=esp2p[64 * half:64 * half + 64, :],
                        rhs=b[64 * half:64 * half + 64, :],
                        start=False, stop=True)
                    t = sb.tile([128, NPC], bf16, tag=f"sps{tag}{2 * m + half}")
                    nc.vector.tensor_copy(out=t, in_=sp)
                    spread.append(t)
            return spread

        qsp = rope_spread(wq, "q")
        ksp = rope_spread(wk, "k")

        # ---- V_aug [256, 408] untransposed (+ ones cols via K=1 matmul) ----
        vau = []
        for i in range(2):
            ps = gpt([128, CAUG])
            for k in range(3):
                nc.tensor.matmul(
                    out=ps,
                    lhsT=xT[k][:, 128 * i:128 * i + 128],
                    rhs=wvil[:, k, :],
                    start=(k == 0), stop=False)
            nc.tensor.matmul(
                out=ps, lhsT=onesrow, rhs=vseed,
                start=False, stop=True)
            t = sb.tile([128, CAUG], bf16, tag=f"vau{i}")
            nc.vector.tensor_copy(out=t, in_=ps)
            vau.append(t)

        # ---- scores S^T + exp, per graph-pair.
        # Concurrent row-tiled matmuls land in distinct PSUM banks:
        # head gh -> bank gh%4 (512-col block), col 64*(gh//4), rows 64*g01.
        def scol(gh):
            return 512 * (gh % 4) + 64 * (gh // 4)

        expst = []
        for pair in range(2):
            stp = ps_att.tile([128, 4 * 512], f32, tag="stps")
            for gh in range(GH):
                tilei, slot = divmod(gh, 4)
                lo = 32 * slot
                for g01 in range(2):
                    g = 2 * pair + g01
                    nc.tensor.matmul(
                        out=stp[64 * g01:64 * g01 + 64, scol(gh):scol(gh) + 64],
                        lhsT=ksp[tilei][lo:lo + 16, 64 * g:64 * g + 64],
                        rhs=qsp[tilei][lo:lo + 16, 64 * g:64 * g + 64],
                        start=True, stop=True,
                        tile_position=(lo, 64 * g01))
            et = sb.tile([128, 4 * 512], bf16, tag=f"expst{pair}")
            for b in range(4):
                nc.scalar.activation(
                    out=et[:, 512 * b:512 * b + 384],
                    in_=stp[:, 512 * b:512 * b + 384],
                    func=AF.Exp, scale=0.25)
            expst.append(et)

        # ---- AV (+den row) per (pair, quad): two [128,64] psum tiles so
        # concurrent graph parities use distinct banks. avsb columns land in
        # natural node order: col = 256*qd + 128*pair + 64*g01 + i64. ----
        avsb = sb.tile([128, 6 * 256], f32, tag="avsb")
        onrm = [[None] * 6 for _ in range(2)]
        ysb = []
        for pair in range(2):
            denps = gpt([GH, 128])
            for qd in range(6):
                avt = [spt([128, 64]) for _ in range(2)]
                nc.vector.memset(avt[0], 0.0)
                nc.vector.memset(avt[1], 0.0)
                for a in range(4):
                    gh = 4 * qd + a
                    for g01 in range(2):
                        nc.tensor.matmul(
                            out=avt[g01][32 * a:32 * a + VW, :],
                            lhsT=vau[pair][64 * g01:64 * g01 + 64,
                                           VW * gh:VW * gh + VW],
                            rhs=expst[pair][64 * g01:64 * g01 + 64,
                                            scol(gh):scol(gh) + 64],
                            start=True, stop=True,
                            tile_position=(64 * g01, 32 * a))
                cbase = 256 * qd + 128 * pair
                for g01 in range(2):
                    nc.vector.tensor_copy(
                        out=avsb[:, cbase + 64 * g01:cbase + 64 * g01 + 64],
                        in_=avt[g01])
                # den rows (32a+16) -> denps rows 4qd+a via tiny f32 matmul
                # (full [24,128] write, 4 nonzero rows, PSUM-accumulated)
                nc.tensor.matmul(
                    out=denps,
                    lhsT=esel[:, GH * qd:GH * qd + GH],
                    rhs=avsb[:, cbase:cbase + 128],
                    start=(qd == 0), stop=(qd == 5))

            # ---- normalize + project this pair (overlaps other pair's AV) --
            rden = sb.tile([GH, 128], f32, tag=f"rden{pair}", name="rden")
            nc.vector.reciprocal(out=rden, in_=denps)
            rdenb = sb.tile([GH, 128], bf16, tag=f"rdenb{pair}", name="rdenb")
            nc.vector.tensor_copy(out=rdenb, in_=rden)
            yps = gpt([128, C])
            for qd in range(6):
                rdsp = spt([128, 128])
                nc.tensor.matmul(
                    out=rdsp, lhsT=bsel[:, 128 * qd:128 * qd + 128],
                    rhs=rdenb, start=True, stop=True)
                ot = sb.tile([128, 128], bf16, tag=f"onrm{pair}_{qd}", name="onrm")
                cbase = 256 * qd + 128 * pair
                nc.vector.tensor_mul(
                    out=ot, in0=avsb[:, cbase:cbase + 128], in1=rdsp)
                onrm[pair][qd] = ot
                nc.tensor.matmul(
                    out=yps, lhsT=ot, rhs=wo_sp[:, C * qd:C * qd + C],
                    start=(qd == 0), stop=(qd == 5))
            yt = sb.tile([128, C], f32, tag=f"ysb{pair}", name="ysb")
            nc.vector.tensor_copy(out=yt, in_=yps)
            nc.sync.dma_start(
                out=y_d[128 * pair:128 * pair + 128, :], in_=yt)
            ysb.append(yt)

    nc.compile()
    return nc


def _get_nc():
    if "nc" not in _CACHE:
        _CACHE["nc"] = _build_nc()
    return _CACHE["nc"]


def make_in_maps(inputs):
    x = np.asarray(inputs["x"], np.float32)
    pos = np.asarray(inputs["pos"], np.float32)
    prep = _host_prep(np.asarray(inputs["Wq"], np.float32),
                      np.asarray(inputs["Wk"], np.float32),
                      np.asarray(inputs["Wv"], np.float32),
                      np.asarray(inputs["Wo"], np.float32),
                      np.asarray(inputs["rope_freqs"], np.float32))
    in_maps = []
    for c in range(NCORES):
        sl = slice(c * NPC, (c + 1) * NPC)
        m = dict(prep)
        xs = x[sl]
        m["x"] = np.ascontiguousarray(
            xs.reshape(2, 128, C).transpose(1, 0, 2).reshape(128, 2 * C)
            .astype(BF))
        pT = np.ascontiguousarray(pos[sl].T.astype(np.float32))
        ptx = np.zeros((4, 2 * NPC), np.float32)
        ptx[:3, :NPC] = pT
        ptx[:3, NPC:] = pT
        ptx[3, NPC:] = 1.0
        m["posTx"] = ptx
        in_maps.append(m)
    return in_maps


def kernel(**inputs):
    from concourse.bass_utils import run_bass_kernel_spmd

    in_maps = make_in_maps(inputs)

    nc = _get_nc()
    res = run_bass_kernel_spmd(nc, in_maps, core_ids=list(range(NCORES)))
    out = np.concatenate([res.results[c]["y"] for c in range(NCORES)], axis=0)
    return out.astype(np.float32)


# revision 12
# speedup vs baseline: 1.7737x; 1.3519x over previous
"""PlatonicConv (graph-mode attention) Trainium2 Bass kernel.

Math (per graph of 64 fully-connected nodes, 24 group-heads of dim 16):
  q/k/v = x @ W; RoPE(q, k) from pos; S = q.k^T/4; softmax over dst;
  out = A @ v; y = out @ Wo.  32 graphs -> data-parallel over 8 cores.

v3 design (per core: 4 graphs, 256 nodes), all matmul operands bf16
(full PE rate at any free dim; fp32/f32r are 4x slower below 256 cols):
  * x^T via PE transposes; q/k projected transposed ([feature, node]).
  * RoPE pair-swap folds into spread matrices: rot_spread accumulates
    esp@(q*cos) + espp@(q*sin) in PSUM -- no Wqp/Wkp weights.
  * theta range-reduction: two chained add_range_wrap custom-DVE ops
    (valid for |theta| < 4.5pi; actual max ~10.8), one Sin table.
  * Q/K spreads use a parity-interleaved layout (graph parity g01 at
    16-row offset inside each 32-slot) so ONE block-diagonal matmul
    computes both parities' scores: 48 score matmuls instead of 96.
  * Softmax is max-free (scores O(1) by construction); denominators are
    a 17th row of each AV matmul via an interleaved ones-column in V,
    extracted/broadcast with tiny PE matmuls, reciprocal_approx_fast.
  * AV output stays in spread layout; Wo rows are pre-spread host-side
    (wo_sp); final y lands in natural node order -> dense DRAM writes.
  * DMA queues: sync HWDGE = x/wq/wvil/y0; scalar HWDGE = small consts
    + y1 (keeps the Sin/Exp activations unblocked); gpsimd SWDGE =
    everything else (wk, wo_sp, spread/selector matrices).
"""

import numpy as np
import ml_dtypes

G = 12
H = 2
D = 16
GH = 24          # G * H group-heads
C = 384          # in/emb/out channels
NG = 32          # graphs
NPG = 64         # nodes per graph
N = NG * NPG
NCORES = 8
GPC = NG // NCORES   # graphs per core = 4
NPC = GPC * NPG      # nodes per core = 256
VW = 17              # V block width (16 + ones col)
CAUG = GH * VW       # 408

BF = ml_dtypes.bfloat16

_CACHE = {}


def _host_prep(Wq, Wk, Wv, Wo, rope_freqs):
    f32 = np.float32

    def pack(w):
        # [384, cols] -> [128, 3*cols]: row p = concat_s w[128 s + p]
        cols = w.shape[1]
        return np.ascontiguousarray(
            w.reshape(3, 128, cols).transpose(1, 0, 2).reshape(128, 3 * cols)
            .astype(BF))

    # V interleaved with a ones column per head: block j = [Wv head j | 0]
    Wvil = np.zeros((C, CAUG), f32)
    for j in range(GH):
        Wvil[:, VW * j:VW * j + 16] = Wv[:, 16 * j:16 * j + 16]
    vseed = np.zeros((1, CAUG), f32)
    vseed[0, VW * np.arange(GH) + 16] = 1.0

    # theta pattern [3, 128] for COMPACT layout rows (dup to both halves):
    # row r=16m+d of a 64-block -> head h=m%2, pair w=d//2
    fr = rope_freqs.astype(f32)            # [3, 2, 8]
    fpat = np.zeros((3, 128), f32)
    for r in range(128):
        rr = r % 64
        fpat[:, r] = fr[:, (rr // 16) % 2, (rr % 16) // 2]

    # spread matrices, parity-interleaved: compact row k (16/head) of a
    # 64-row half -> slot row 32s + 16*P + j for graph parity P. Variant
    # 2P   = plain (multiplies q*cos), variant 2P+1 = pair-swap signed
    # (multiplies q*sin). Two stacked copies for odd 64-row slabs.
    espx = np.zeros((128, 4, 128), f32)
    for k in range(64):
        s, j = divmod(k, 16)
        jp = j + 1 if j % 2 == 0 else j - 1
        sg = 1.0 if j % 2 == 0 else -1.0
        for P in range(2):
            espx[k, 2 * P, 32 * s + 16 * P + j] = 1.0
            espx[64 + k, 2 * P, 32 * s + 16 * P + j] = 1.0
            espx[k, 2 * P + 1, 32 * s + 16 * P + jp] = sg
            espx[64 + k, 2 * P + 1, 32 * s + 16 * P + jp] = sg

    # den extract: per quad qd, spread row 32a+16 -> out row 4qd+a of a
    # [24,128] PSUM accumulation (out base partition must be 32-aligned,
    # so each quad writes the full 24 rows, 4 nonzero)
    esel = np.zeros((128, 6 * GH), f32)
    for qd in range(6):
        for a in range(4):
            esel[32 * a + 16, GH * qd + 4 * qd + a] = 1.0

    # den broadcast: rden row gh -> 16 spread rows of its quad block
    bsel = np.zeros((GH, 6 * 128), f32)
    for gh in range(GH):
        qd, a = divmod(gh, 4)
        bsel[gh, 128 * qd + 32 * a:128 * qd + 32 * a + 16] = 1.0

    # Wo with rows pre-spread to the AV output layout (den/pad rows = 0)
    wo_sp = np.zeros((128, 6 * C), f32)
    for gh in range(GH):
        qd, a = divmod(gh, 4)
        wo_sp[32 * a:32 * a + 16, C * qd:C * qd + C] = Wo[16 * gh:16 * gh + 16]

    return dict(
        wq=pack(Wq), wk=pack(Wk), wvil=pack(Wvil),
        wo_sp=wo_sp.astype(BF), vseed=vseed.astype(BF),
        fpat=fpat,
        espx=np.ascontiguousarray(espx.reshape(128, 4 * 128)).astype(BF),
        esel=esel.astype(BF), bsel=bsel.astype(BF),
        onesrow=np.ones((1, 128), BF), ident=np.eye(128, dtype=BF),
    )


def _build_nc():
    import concourse.bacc as bacc
    import concourse.tile as tile
    import concourse.mybir as mybir
    from contextlib import ExitStack

    f32 = mybir.dt.float32
    bf16 = mybir.dt.bfloat16
    AF = mybir.ActivationFunctionType

    nc = bacc.Bacc("TRN2", target_bir_lowering=False)

    x_d = nc.dram_tensor("x", [128, 2 * C], bf16, kind="ExternalInput")
    posT_d = nc.dram_tensor("posT", [3, NPC], f32, kind="ExternalInput")
    wq_d = nc.dram_tensor("wq", [128, 3 * C], bf16, kind="ExternalInput")
    wk_d = nc.dram_tensor("wk", [128, 3 * C], bf16, kind="ExternalInput")
    wvil_d = nc.dram_tensor("wvil", [128, 3 * CAUG], bf16, kind="ExternalInput")
    wo_sp_d = nc.dram_tensor("wo_sp", [128, 6 * C], bf16, kind="ExternalInput")
    vseed_d = nc.dram_tensor("vseed", [1, CAUG], bf16, kind="ExternalInput")
    fpat_d = nc.dram_tensor("fpat", [3, 128], f32, kind="ExternalInput")
    espx_d = nc.dram_tensor("espx", [128, 4 * 128], bf16, kind="ExternalInput")
    esel_d = nc.dram_tensor("esel", [128, 6 * GH], bf16, kind="ExternalInput")
    bsel_d = nc.dram_tensor("bsel", [GH, 6 * 128], bf16, kind="ExternalInput")
    ones_d = nc.dram_tensor("onesrow", [1, 128], bf16, kind="ExternalInput")
    ident_d = nc.dram_tensor("ident", [128, 128], bf16, kind="ExternalInput")
    y_d = nc.dram_tensor("y", [NPC, C], f32, kind="ExternalOutput")

    ctx = ExitStack()
    with tile.TileContext(nc) as tc, ctx:
        consts = ctx.enter_context(tc.tile_pool(name="consts", bufs=1))
        wpool = ctx.enter_context(tc.tile_pool(name="weights", bufs=1))
        sb = ctx.enter_context(tc.tile_pool(name="sbuf", bufs=1))
        # 2+2+4 PSUM banks: gp (proj/theta/vau/den/bcast/y), sp (spread/AV),
        # att (score tiles; bank gh%4 for 4-way concurrent row groups)
        ps_gp = ctx.enter_context(tc.tile_pool(name="ps_gp", bufs=2, space="PSUM"))
        ps_sp = ctx.enter_context(tc.tile_pool(name="ps_sp", bufs=2, space="PSUM"))
        ps_att = ctx.enter_context(tc.tile_pool(name="ps_att", bufs=1, space="PSUM"))

        def gpt(shape, dt=f32):
            return ps_gp.tile(shape, dt, tag="pp", name="pp")

        def spt(shape, dt=f32):
            return ps_sp.tile(shape, dt, tag="sp", name="sp")

        # ---- inputs; x/wq/wvil on sync HWDGE, tiny consts on scalar HWDGE,
        # the rest on gpsimd SWDGE so the scalar engine stays free ----
        xsb = sb.tile([128, 2, C], bf16, tag="x")
        nc.sync.dma_start(out=xsb, in_=x_d.rearrange("p (s e) -> p s e", s=2))
        ident = consts.tile([128, 128], bf16, tag="ident")
        nc.scalar.dma_start(out=ident, in_=ident_d[:])
        posT = consts.tile([3, NPC], f32, tag="posT")
        nc.scalar.dma_start(out=posT, in_=posT_d[:])
        fpat = consts.tile([3, 128], f32, tag="fpat")
        nc.scalar.dma_start(out=fpat, in_=fpat_d[:])
        espx = consts.tile([128, 4, 128], bf16, tag="espx")
        nc.gpsimd.dma_start(out=espx, in_=espx_d.rearrange("p (v e) -> p v e", v=4))
        esel = consts.tile([128, 6 * GH], bf16, tag="esel")
        nc.gpsimd.dma_start(out=esel, in_=esel_d[:])
        bsel = consts.tile([GH, 6 * 128], bf16, tag="bsel")
        nc.gpsimd.dma_start(out=bsel, in_=bsel_d[:])
        vseed = consts.tile([1, CAUG], bf16, tag="vseed")
        nc.gpsimd.dma_start(out=vseed, in_=vseed_d[:])
        onesrow = consts.tile([1, 128], bf16, tag="ones")
        nc.gpsimd.dma_start(out=onesrow, in_=ones_d[:])

        wq = wpool.tile([128, 3, C], bf16, tag="wq")
        nc.sync.dma_start(out=wq, in_=wq_d.rearrange("p (s e) -> p s e", s=3))
        wvil = wpool.tile([128, 3, CAUG], bf16, tag="wvil")
        nc.sync.dma_start(out=wvil, in_=wvil_d.rearrange("p (s e) -> p s e", s=3))
        wk = wpool.tile([128, 3, C], bf16, tag="wk")
        nc.gpsimd.dma_start(out=wk, in_=wk_d.rearrange("p (s e) -> p s e", s=3))
        wo_sp = wpool.tile([128, 6 * C], bf16, tag="wo_sp")
        nc.gpsimd.dma_start(out=wo_sp, in_=wo_sp_d[:])

        # ---- X^T [384, 256] via PE transposes (bf16) ----
        xT = []
        for j in range(3):
            t = sb.tile([128, NPC], bf16, tag=f"xT{j}")
            for i in range(2):
                pst = gpt([128, 128], bf16)
                nc.tensor.transpose(
                    out=pst, in_=xsb[:, i, 128 * j:128 * j + 128], identity=ident)
                nc.vector.tensor_copy(out=t[:, 128 * i:128 * i + 128], in_=pst)
            xT.append(t)

        # ---- theta [128, 256]; range-reduce via 2 chained add_range_wrap
        # (safe for |theta| < 4.5pi); Sin table gives both sin and cos ----
        PI = float(np.pi)
        thps = gpt([128, NPC])
        nc.tensor.matmul(
            out=thps, lhsT=fpat,
            rhs=posT,
            start=True, stop=True)
        cpat = sb.tile([128, NPC], f32, tag="cpat")
        spat = sb.tile([128, NPC], f32, tag="spat")
        for (tgt, shift) in ((spat, 0.0), (cpat, PI / 2)):
            w1 = sb.tile([128, NPC], f32, tag=f"w1{shift}", name="w1")
            nc.vector.add_range_wrap(out=w1, in_=thps, shift=shift,
                                     bound=PI, period=2 * PI)
            w2 = sb.tile([128, NPC], f32, tag=f"w2{shift}", name="w2")
            nc.vector.add_range_wrap(out=w2, in_=w1, shift=0.0,
                                     bound=PI, period=2 * PI)
            nc.scalar.activation(out=tgt, in_=w2, func=AF.Sin)

        # ---- projections (transposed) + RoPE + parity-interleaved spread.
        # qz[t] [128,128]: slot rows 32s+16P+j, cols (pair, src i) -- both
        # parities' q stacked per column.  kz[t] [128, (pr, P, i)]: same rows,
        # block-diag in data (parity P data only in P's 16-row sub-slot). ----
        def proj_m(w, m):
            ps = gpt([128, NPC])
            for k in range(3):
                nc.tensor.matmul(
                    out=ps,
                    lhsT=w[:, k, 128 * m:128 * m + 128],
                    rhs=xT[k],
                    start=(k == 0), stop=(k == 2))
            return ps

        qz, kz = [], []
        ncopy = [0]

        def copy_alt(out, in_):
            # alternate psum->sbuf copies between vector and scalar engines
            ncopy[0] += 1
            if ncopy[0] % 2:
                nc.vector.tensor_copy(out=out, in_=in_)
            else:
                nc.scalar.activation(out=out, in_=in_, func=AF.Copy)

        for m in range(3):
            for (w, out_tiles, isq, tg) in ((wq, qz, True, "q"), (wk, kz, False, "k")):
                qt = proj_m(w, m)
                a = sb.tile([128, 2, 2, 64], bf16, tag=f"ra{tg}{m}")
                b = sb.tile([128, 2, 2, 64], bf16, tag=f"rb{tg}{m}")
                nc.vector.tensor_mul(out=a, in0=qt, in1=cpat)
                nc.vector.tensor_mul(out=b, in0=qt, in1=spat)
                for half in range(2):
                    hs = slice(64 * half, 64 * half + 64)
                    if isq:
                        sp = spt([128, 128])
                        for P in range(2):
                            nc.tensor.matmul(
                                out=sp, lhsT=espx[hs, 2 * P, :],
                                rhs=a[hs, :, P, :],
                                start=(P == 0), stop=False)
                            nc.tensor.matmul(
                                out=sp, lhsT=espx[hs, 2 * P + 1, :],
                                rhs=b[hs, :, P, :],
                                start=False, stop=(P == 1))
                        t = sb.tile([128, 128], bf16, tag=f"sps{tg}{2 * m + half}")
                    else:
                        sp = spt([128, 2, 2, 64])
                        for P in range(2):
                            nc.tensor.matmul(
                                out=sp[:, :, P, :], lhsT=espx[hs, 2 * P, :],
                                rhs=a[hs, :, P, :],
                                start=True, stop=False)
                            nc.tensor.matmul(
                                out=sp[:, :, P, :], lhsT=espx[hs, 2 * P + 1, :],
                                rhs=b[hs, :, P, :],
                                start=False, stop=True)
                        t = sb.tile([128, 2, 2, 64], bf16,
                                    tag=f"sps{tg}{2 * m + half}")
                    copy_alt(t, sp)
                    out_tiles.append(t)

        # ---- V_aug [256, 408] untransposed (+ ones cols via K=1 matmul) ----
        vau = []
        for i in range(2):
            ps = gpt([128, CAUG])
            for k in range(3):
                nc.tensor.matmul(
                    out=ps,
                    lhsT=xT[k][:, 128 * i:128 * i + 128],
                    rhs=wvil[:, k, :],
                    start=(k == 0), stop=False)
            nc.tensor.matmul(
                out=ps, lhsT=onesrow, rhs=vseed,
                start=False, stop=True)
            t = sb.tile([128, CAUG], bf16, tag=f"vau{i}")
            copy_alt(t, ps)
            vau.append(t)

        # ---- scores S^T + exp, per graph-pair: ONE block-diag matmul per
        # (gh, pair) covers both parities.  head gh -> bank gh%4 (512-col
        # block), col 64*(gh//4); rows 64*P via the kz data layout. ----
        def scol(gh):
            return 512 * (gh % 4) + 64 * (gh // 4)

        expst = []
        for pair in range(2):
            stp = ps_att.tile([128, 4 * 512], f32, tag="stps")
            for gh in range(GH):
                tilei, slot = divmod(gh, 4)
                lo = 32 * slot
                nc.tensor.matmul(
                    out=stp[:, scol(gh):scol(gh) + 64],
                    lhsT=kz[tilei][lo:lo + 32, pair, :, :],
                    rhs=qz[tilei][lo:lo + 32, 64 * pair:64 * pair + 64],
                    start=True, stop=True,
                    tile_position=(lo, 0))
            et = sb.tile([128, 4 * 512], bf16, tag=f"expst{pair}")
            for b in range(4):
                nc.scalar.activation(
                    out=et[:, 512 * b:512 * b + 384],
                    in_=stp[:, 512 * b:512 * b + 384],
                    func=AF.Exp, scale=0.25)
            expst.append(et)

        # ---- AV (+den row) per (pair, quad): two [128,64] psum tiles so
        # concurrent graph parities use distinct banks. avsb columns land in
        # natural node order: col = 256*qd + 128*pair + 64*g01 + i64.
        # (no memsets: psum slots hold finite data from the spread phase,
        # and pad rows are zero-masked by rdsp/wo_sp downstream) ----
        avsb = sb.tile([128, 6 * 256], bf16, tag="avsb")
        for pair in range(2):
            denps = gpt([GH, 128])
            for qd in range(6):
                avt = [spt([128, 64]) for _ in range(2)]
                for a in range(4):
                    gh = 4 * qd + a
                    for g01 in range(2):
                        nc.tensor.matmul(
                            out=avt[g01][32 * a:32 * a + VW, :],
                            lhsT=vau[pair][64 * g01:64 * g01 + 64,
                                           VW * gh:VW * gh + VW],
                            rhs=expst[pair][64 * g01:64 * g01 + 64,
                                            scol(gh):scol(gh) + 64],
                            start=True, stop=True,
                            tile_position=(64 * g01, 32 * a))
                cbase = 256 * qd + 128 * pair
                for g01 in range(2):
                    copy_alt(avsb[:, cbase + 64 * g01:cbase + 64 * g01 + 64],
                             avt[g01])
                # den rows (32a+16) -> denps rows 4qd+a (bf16 matmul,
                # full [24,128] write with 4 nonzero rows, accumulated)
                nc.tensor.matmul(
                    out=denps,
                    lhsT=esel[:, GH * qd:GH * qd + GH],
                    rhs=avsb[:, cbase:cbase + 128],
                    start=(qd == 0), stop=(qd == 5))

            # ---- normalize + project this pair (overlaps other pair's AV) --
            rden = sb.tile([GH, 128], f32, tag=f"rden{pair}", name="rden")
            nc.vector.reciprocal_approx_fast(out=rden, in_=denps)
            rdenb = sb.tile([GH, 128], bf16, tag=f"rdenb{pair}", name="rdenb")
            nc.vector.tensor_copy(out=rdenb, in_=rden)
            yps = gpt([128, C])
            for qd in range(6):
                rdsp = ps_att.tile([128, 128], f32, tag="stps", name="rdsp")
                nc.tensor.matmul(
                    out=rdsp, lhsT=bsel[:, 128 * qd:128 * qd + 128],
                    rhs=rdenb, start=True, stop=True)
                ot = sb.tile([128, 128], bf16, tag=f"onrm{pair}_{qd}", name="onrm")
                cbase = 256 * qd + 128 * pair
                nc.vector.tensor_mul(
                    out=ot, in0=avsb[:, cbase:cbase + 128], in1=rdsp)
                nc.tensor.matmul(
                    out=yps, lhsT=ot, rhs=wo_sp[:, C * qd:C * qd + C],
                    start=(qd == 0), stop=(qd == 5))
            yt = sb.tile([128, C], f32, tag=f"ysb{pair}", name="ysb")
            nc.vector.tensor_copy(out=yt, in_=yps)
            (nc.sync if pair == 0 else nc.scalar).dma_start(
                out=y_d[128 * pair:128 * pair + 128, :], in_=yt)

    nc.compile()
    return nc


def _get_nc():
    if "nc" not in _CACHE:
        _CACHE["nc"] = _build_nc()
    return _CACHE["nc"]


def make_in_maps(inputs):
    x = np.asarray(inputs["x"], np.float32)
    pos = np.asarray(inputs["pos"], np.float32)
    prep = _host_prep(np.asarray(inputs["Wq"], np.float32),
                      np.asarray(inputs["Wk"], np.float32),
                      np.asarray(inputs["Wv"], np.float32),
                      np.asarray(inputs["Wo"], np.float32),
                      np.asarray(inputs["rope_freqs"], np.float32))
    in_maps = []
    for c in range(NCORES):
        sl = slice(c * NPC, (c + 1) * NPC)
        m = dict(prep)
        xs = x[sl]
        m["x"] = np.ascontiguousarray(
            xs.reshape(2, 128, C).transpose(1, 0, 2).reshape(128, 2 * C)
            .astype(BF))
        m["posT"] = np.ascontiguousarray(pos[sl].T.astype(np.float32))
        in_maps.append(m)
    return in_maps


def kernel(**inputs):
    from concourse.bass_utils import run_bass_kernel_spmd

    in_maps = make_in_maps(inputs)

    nc = _get_nc()
    res = run_bass_kernel_spmd(nc, in_maps, core_ids=list(range(NCORES)))
    out = np.concatenate([res.results[c]["y"] for c in range(NCORES)], axis=0)
    return out.astype(np.float32)


# revision 15
# speedup vs baseline: 1.8826x; 1.0614x over previous
"""PlatonicConv (graph-mode attention) Trainium2 Bass kernel.

Math (per graph of 64 fully-connected nodes, 24 group-heads of dim 16):
  q/k/v = x @ W; RoPE(q, k) from pos; S = q.k^T/4; softmax over dst;
  out = A @ v; y = out @ Wo.  32 graphs -> data-parallel over 8 cores.

v3 design (per core: 4 graphs, 256 nodes), all matmul operands bf16
(full PE rate at any free dim; fp32/f32r are 4x slower below 256 cols):
  * x^T via PE transposes; q/k projected transposed ([feature, node]).
  * RoPE pair-swap folds into spread matrices: rot_spread accumulates
    esp@(q*cos) + espp@(q*sin) in PSUM -- no Wqp/Wkp weights.
  * theta range-reduction: two chained add_range_wrap custom-DVE ops
    (valid for |theta| < 4.5pi; actual max ~10.8), one Sin table.
  * Q/K spreads use a parity-interleaved layout (graph parity g01 at
    16-row offset inside each 32-slot) so ONE block-diagonal matmul
    computes both parities' scores: 48 score matmuls instead of 96.
  * Softmax is max-free (scores O(1) by construction); denominators are
    a 17th row of each AV matmul via an interleaved ones-column in V,
    extracted/broadcast with tiny PE matmuls, reciprocal_approx_fast.
  * AV output stays in spread layout; Wo rows are pre-spread host-side
    (wo_sp); final y lands in natural node order -> dense DRAM writes.
  * DMA queues: sync HWDGE = x/wq/wvil/y0; scalar HWDGE = small consts
    + y1 (keeps the Sin/Exp activations unblocked); gpsimd SWDGE =
    everything else (wk, wo_sp, spread/selector matrices).
"""

import numpy as np
import ml_dtypes

G = 12
H = 2
D = 16
GH = 24          # G * H group-heads
C = 384          # in/emb/out channels
NG = 32          # graphs
NPG = 64         # nodes per graph
N = NG * NPG
NCORES = 8
GPC = NG // NCORES   # graphs per core = 4
NPC = GPC * NPG      # nodes per core = 256
VW = 17              # V block width (16 + ones col)
CAUG = GH * VW       # 408

BF = ml_dtypes.bfloat16

_CACHE = {}


def _host_prep(Wq, Wk, Wv, Wo, rope_freqs):
    f32 = np.float32

    def pack(w):
        # [384, cols] -> [128, 3*cols]: row p = concat_s w[128 s + p]
        cols = w.shape[1]
        return np.ascontiguousarray(
            w.reshape(3, 128, cols).transpose(1, 0, 2).reshape(128, 3 * cols)
            .astype(BF))

    # V interleaved with a ones column per head: block j = [Wv head j | 0]
    Wvil = np.zeros((C, CAUG), f32)
    for j in range(GH):
        Wvil[:, VW * j:VW * j + 16] = Wv[:, 16 * j:16 * j + 16]
    vseed = np.zeros((1, CAUG), f32)
    vseed[0, VW * np.arange(GH) + 16] = 1.0

    # theta pattern [3, 128] for COMPACT layout rows (dup to both halves):
    # row r=16m+d of a 64-block -> head h=m%2, pair w=d//2
    fr = rope_freqs.astype(f32)            # [3, 2, 8]
    fpat = np.zeros((3, 128), f32)
    for r in range(128):
        rr = r % 64
        fpat[:, r] = fr[:, (rr // 16) % 2, (rr % 16) // 2]

    # spread matrices, parity-interleaved: compact row k (16/head) of a
    # 64-row half -> slot row 32s + 16*P + j for graph parity P. Variant
    # 2P   = plain (multiplies q*cos), variant 2P+1 = pair-swap signed
    # (multiplies q*sin). Two stacked copies for odd 64-row slabs.
    espx = np.zeros((128, 4, 128), f32)
    for k in range(64):
        s, j = divmod(k, 16)
        jp = j + 1 if j % 2 == 0 else j - 1
        sg = 1.0 if j % 2 == 0 else -1.0
        for P in range(2):
            espx[k, 2 * P, 32 * s + 16 * P + j] = 1.0
            espx[64 + k, 2 * P, 32 * s + 16 * P + j] = 1.0
            espx[k, 2 * P + 1, 32 * s + 16 * P + jp] = sg
            espx[64 + k, 2 * P + 1, 32 * s + 16 * P + jp] = sg

    # den extract: per quad qd, spread row 32a+16 -> out row 4qd+a of a
    # [24,128] PSUM accumulation (out base partition must be 32-aligned,
    # so each quad writes the full 24 rows, 4 nonzero)
    esel = np.zeros((128, 6 * GH), f32)
    for qd in range(6):
        for a in range(4):
            esel[32 * a + 16, GH * qd + 4 * qd + a] = 1.0

    # den broadcast: rden row gh -> 16 spread rows of its quad block
    bsel = np.zeros((GH, 6 * 128), f32)
    for gh in range(GH):
        qd, a = divmod(gh, 4)
        bsel[gh, 128 * qd + 32 * a:128 * qd + 32 * a + 16] = 1.0

    # Wo with rows pre-spread to the AV output layout (den/pad rows = 0)
    wo_sp = np.zeros((128, 6 * C), f32)
    for gh in range(GH):
        qd, a = divmod(gh, 4)
        wo_sp[32 * a:32 * a + 16, C * qd:C * qd + C] = Wo[16 * gh:16 * gh + 16]

    return dict(
        wq=pack(Wq), wk=pack(Wk), wvil=pack(Wvil),
        wo_sp=wo_sp.astype(BF), vseed=vseed.astype(BF),
        fpat=fpat,
        espx=np.ascontiguousarray(espx.reshape(128, 4 * 128)).astype(BF),
        esel=esel.astype(BF), bsel=bsel.astype(BF),
        onesrow=np.ones((1, 128), BF), ident=np.eye(128, dtype=BF),
    )


def _build_nc():
    import concourse.bacc as bacc
    import concourse.tile as tile
    import concourse.mybir as mybir
    from contextlib import ExitStack

    f32 = mybir.dt.float32
    bf16 = mybir.dt.bfloat16
    AF = mybir.ActivationFunctionType

    nc = bacc.Bacc("TRN2", target_bir_lowering=False)

    x_d = nc.dram_tensor("x", [128, 2 * C], bf16, kind="ExternalInput")
    posT_d = nc.dram_tensor("posT", [3, NPC], f32, kind="ExternalInput")
    wq_d = nc.dram_tensor("wq", [128, 3 * C], bf16, kind="ExternalInput")
    wk_d = nc.dram_tensor("wk", [128, 3 * C], bf16, kind="ExternalInput")
    wvil_d = nc.dram_tensor("wvil", [128, 3 * CAUG], bf16, kind="ExternalInput")
    wo_sp_d = nc.dram_tensor("wo_sp", [128, 6 * C], bf16, kind="ExternalInput")
    vseed_d = nc.dram_tensor("vseed", [1, CAUG], bf16, kind="ExternalInput")
    fpat_d = nc.dram_tensor("fpat", [3, 128], f32, kind="ExternalInput")
    espx_d = nc.dram_tensor("espx", [128, 4 * 128], bf16, kind="ExternalInput")
    esel_d = nc.dram_tensor("esel", [128, 6 * GH], bf16, kind="ExternalInput")
    bsel_d = nc.dram_tensor("bsel", [GH, 6 * 128], bf16, kind="ExternalInput")
    ones_d = nc.dram_tensor("onesrow", [1, 128], bf16, kind="ExternalInput")
    ident_d = nc.dram_tensor("ident", [128, 128], bf16, kind="ExternalInput")
    y_d = nc.dram_tensor("y", [NPC, C], f32, kind="ExternalOutput")

    ctx = ExitStack()
    with tile.TileContext(nc) as tc, ctx:
        consts = ctx.enter_context(tc.tile_pool(name="consts", bufs=1))
        wpool = ctx.enter_context(tc.tile_pool(name="weights", bufs=1))
        sb = ctx.enter_context(tc.tile_pool(name="sbuf", bufs=1))
        # 2+2+4 PSUM banks: gp (proj/theta/vau/den/bcast/y), sp (spread/AV),
        # att (score tiles; bank gh%4 for 4-way concurrent row groups)
        ps_gp = ctx.enter_context(tc.tile_pool(name="ps_gp", bufs=2, space="PSUM"))
        ps_sp = ctx.enter_context(tc.tile_pool(name="ps_sp", bufs=2, space="PSUM"))
        ps_att = ctx.enter_context(tc.tile_pool(name="ps_att", bufs=1, space="PSUM"))

        def gpt(shape, dt=f32):
            return ps_gp.tile(shape, dt, tag="pp", name="pp")

        def spt(shape, dt=f32):
            return ps_sp.tile(shape, dt, tag="sp", name="sp")

        # ---- inputs; x/wq/wvil on sync HWDGE, tiny consts on scalar HWDGE,
        # the rest on gpsimd SWDGE so the scalar engine stays free ----
        xsb = sb.tile([128, 2, C], bf16, tag="x")
        nc.sync.dma_start(out=xsb, in_=x_d.rearrange("p (s e) -> p s e", s=2))
        ident = consts.tile([128, 128], bf16, tag="ident")
        nc.scalar.dma_start(out=ident, in_=ident_d[:])
        posT = consts.tile([3, NPC], f32, tag="posT")
        nc.scalar.dma_start(out=posT, in_=posT_d[:])
        fpat = consts.tile([3, 128], f32, tag="fpat")
        nc.scalar.dma_start(out=fpat, in_=fpat_d[:])
        espx = consts.tile([128, 4, 128], bf16, tag="espx")
        nc.gpsimd.dma_start(out=espx, in_=espx_d.rearrange("p (v e) -> p v e", v=4))
        esel = consts.tile([128, 6 * GH], bf16, tag="esel")
        nc.gpsimd.dma_start(out=esel, in_=esel_d[:])
        bsel = consts.tile([GH, 6 * 128], bf16, tag="bsel")
        nc.gpsimd.dma_start(out=bsel, in_=bsel_d[:])
        vseed = consts.tile([1, CAUG], bf16, tag="vseed")
        nc.gpsimd.dma_start(out=vseed, in_=vseed_d[:])
        onesrow = consts.tile([1, 128], bf16, tag="ones")
        nc.gpsimd.dma_start(out=onesrow, in_=ones_d[:])

        wq = wpool.tile([128, 3, C], bf16, tag="wq")
        nc.sync.dma_start(out=wq, in_=wq_d.rearrange("p (s e) -> p s e", s=3))
        wvil = wpool.tile([128, 3, CAUG], bf16, tag="wvil")
        nc.sync.dma_start(out=wvil, in_=wvil_d.rearrange("p (s e) -> p s e", s=3))
        wk = wpool.tile([128, 3, C], bf16, tag="wk")
        nc.gpsimd.dma_start(out=wk, in_=wk_d.rearrange("p (s e) -> p s e", s=3))
        wo_sp = wpool.tile([128, 6 * C], bf16, tag="wo_sp")
        nc.gpsimd.dma_start(out=wo_sp, in_=wo_sp_d[:])

        # ---- X^T [384, 256] via PE transposes (bf16) ----
        xT = []
        for j in range(3):
            t = sb.tile([128, NPC], bf16, tag=f"xT{j}")
            for i in range(2):
                pst = gpt([128, 128], bf16)
                nc.tensor.transpose(
                    out=pst, in_=xsb[:, i, 128 * j:128 * j + 128], identity=ident)
                nc.vector.tensor_copy(out=t[:, 128 * i:128 * i + 128], in_=pst)
            xT.append(t)

        # ---- theta [128, 256]; range-reduce via 2 chained add_range_wrap
        # (safe for |theta| < 4.5pi); Sin table gives both sin and cos ----
        PI = float(np.pi)
        thps = gpt([128, NPC])
        nc.tensor.matmul(
            out=thps, lhsT=fpat,
            rhs=posT,
            start=True, stop=True)
        cpat = sb.tile([128, NPC], f32, tag="cpat")
        spat = sb.tile([128, NPC], f32, tag="spat")
        for (tgt, shift) in ((spat, 0.0), (cpat, PI / 2)):
            w1 = sb.tile([128, NPC], f32, tag=f"w1{shift}", name="w1")
            nc.vector.add_range_wrap(out=w1, in_=thps, shift=shift,
                                     bound=PI, period=2 * PI)
            w2 = sb.tile([128, NPC], f32, tag=f"w2{shift}", name="w2")
            nc.vector.add_range_wrap(out=w2, in_=w1, shift=0.0,
                                     bound=PI, period=2 * PI)
            nc.scalar.activation(out=tgt, in_=w2, func=AF.Sin)

        # ---- projections (transposed) + RoPE + parity-interleaved spread.
        # qz[t] [128,128]: slot rows 32s+16P+j, cols (pair, src i) -- both
        # parities' q stacked per column.  kz[t] [128, (pr, P, i)]: same rows,
        # block-diag in data (parity P data only in P's 16-row sub-slot). ----
        def proj_m(w, m):
            ps = gpt([128, NPC])
            for k in range(3):
                nc.tensor.matmul(
                    out=ps,
                    lhsT=w[:, k, 128 * m:128 * m + 128],
                    rhs=xT[k],
                    start=(k == 0), stop=(k == 2))
            return ps

        qz, kz = [], []
        ncopy = [0]

        def copy_alt(out, in_):
            # alternate psum->sbuf copies between vector and scalar engines
            ncopy[0] += 1
            if ncopy[0] % 2:
                nc.vector.tensor_copy(out=out, in_=in_)
            else:
                nc.scalar.activation(out=out, in_=in_, func=AF.Copy)

        for m in range(3):
            for (w, out_tiles, isq, tg) in ((wq, qz, True, "q"), (wk, kz, False, "k")):
                qt = proj_m(w, m)
                a = sb.tile([128, 2, 2, 64], bf16, tag=f"ra{tg}{m}")
                b = sb.tile([128, 2, 2, 64], bf16, tag=f"rb{tg}{m}")
                nc.vector.tensor_mul(out=a, in0=qt, in1=cpat)
                nc.vector.tensor_mul(out=b, in0=qt, in1=spat)
                for half in range(2):
                    hs = slice(64 * half, 64 * half + 64)
                    if isq:
                        sp = spt([128, 128])
                        for P in range(2):
                            nc.tensor.matmul(
                                out=sp, lhsT=espx[hs, 2 * P, :],
                                rhs=a[hs, :, P, :],
                                start=(P == 0), stop=False)
                            nc.tensor.matmul(
                                out=sp, lhsT=espx[hs, 2 * P + 1, :],
                                rhs=b[hs, :, P, :],
                                start=False, stop=(P == 1))
                        t = sb.tile([128, 128], bf16, tag=f"sps{tg}{2 * m + half}")
                    else:
                        sp = spt([128, 2, 2, 64])
                        for P in range(2):
                            nc.tensor.matmul(
                                out=sp[:, :, P, :], lhsT=espx[hs, 2 * P, :],
                                rhs=a[hs, :, P, :],
                                start=True, stop=False)
                            nc.tensor.matmul(
                                out=sp[:, :, P, :], lhsT=espx[hs, 2 * P + 1, :],
                                rhs=b[hs, :, P, :],
                                start=False, stop=True)
                        t = sb.tile([128, 2, 2, 64], bf16,
                                    tag=f"sps{tg}{2 * m + half}")
                    copy_alt(t, sp)
                    out_tiles.append(t)

        # ---- V_aug [256, 408] untransposed (+ ones cols via K=1 matmul) ----
        vau = []
        for i in range(2):
            ps = gpt([128, CAUG])
            for k in range(3):
                nc.tensor.matmul(
                    out=ps,
                    lhsT=xT[k][:, 128 * i:128 * i + 128],
                    rhs=wvil[:, k, :],
                    start=(k == 0), stop=False)
            nc.tensor.matmul(
                out=ps, lhsT=onesrow, rhs=vseed,
                start=False, stop=True)
            t = sb.tile([128, CAUG], bf16, tag=f"vau{i}")
            copy_alt(t, ps)
            vau.append(t)

        # ---- scores S^T + exp, per graph-pair: ONE block-diag matmul per
        # (gh, pair) covers both parities.  head gh -> bank gh%4 (512-col
        # block), col 64*(gh//4); rows 64*P via the kz data layout. ----
        def scol(gh):
            return 512 * (gh % 4) + 64 * (gh // 4)

        expst = []
        for pair in range(2):
            stp = ps_att.tile([128, 4 * 512], f32, tag="stps")
            for gh in range(GH):
                tilei, slot = divmod(gh, 4)
                lo = 32 * slot
                nc.tensor.matmul(
                    out=stp[:, scol(gh):scol(gh) + 64],
                    lhsT=kz[tilei][lo:lo + 32, pair, :, :],
                    rhs=qz[tilei][lo:lo + 32, 64 * pair:64 * pair + 64],
                    start=True, stop=True,
                    tile_position=(lo, 0))
            et = sb.tile([128, 4 * 512], bf16, tag=f"expst{pair}")
            # 2 chunks per bank so AV quads 0-2 can start before the
            # bank's later columns (tiles 3-5) are scored
            for b in range(4):
                for ch in range(2):
                    co = 512 * b + 192 * ch
                    nc.scalar.activation(
                        out=et[:, co:co + 192],
                        in_=stp[:, co:co + 192],
                        func=AF.Exp, scale=0.25)
            expst.append(et)

        # ---- AV (+den row) per (pair, quad): two [128,64] psum tiles so
        # concurrent graph parities use distinct banks. avsb columns land in
        # natural node order: col = 256*qd + 128*pair + 64*g01 + i64.
        # (no memsets: psum slots hold finite data from the spread phase,
        # and pad rows are zero-masked by rdsp/wo_sp downstream) ----
        avsb = sb.tile([128, 6 * 256], bf16, tag="avsb")
        for pair in range(2):
            for qd in range(6):
                avt = [spt([128, 64]) for _ in range(2)]
                for a in range(4):
                    gh = 4 * qd + a
                    for g01 in range(2):
                        nc.tensor.matmul(
                            out=avt[g01][32 * a:32 * a + VW, :],
                            lhsT=vau[pair][64 * g01:64 * g01 + 64,
                                           VW * gh:VW * gh + VW],
                            rhs=expst[pair][64 * g01:64 * g01 + 64,
                                            scol(gh):scol(gh) + 64],
                            start=True, stop=True,
                            tile_position=(64 * g01, 32 * a))
                cbase = 256 * qd + 128 * pair
                for g01 in range(2):
                    copy_alt(avsb[:, cbase + 64 * g01:cbase + 64 * g01 + 64],
                             avt[g01])

        # ---- merged tail: den extract (both pairs per quad), reciprocal,
        # broadcast, normalize, project.  den rows (32a+16) -> denps rows
        # 4qd+a (full [24,256] write with 4 nonzero rows, accumulated) ----
        denps = gpt([GH, 2 * 128])
        for qd in range(6):
            nc.tensor.matmul(
                out=denps,
                lhsT=esel[:, GH * qd:GH * qd + GH],
                rhs=avsb[:, 256 * qd:256 * qd + 256],
                start=(qd == 0), stop=(qd == 5))
        rden = sb.tile([GH, 2 * 128], f32, tag="rden")
        nc.vector.reciprocal_approx_fast(out=rden, in_=denps)
        rdenb = sb.tile([GH, 2 * 128], bf16, tag="rdenb")
        nc.vector.tensor_copy(out=rdenb, in_=rden)
        yps = [gpt([128, C]) for _ in range(2)]
        for qd in range(6):
            rdsp = spt([128, 2 * 128])
            nc.tensor.matmul(
                out=rdsp, lhsT=bsel[:, 128 * qd:128 * qd + 128],
                rhs=rdenb, start=True, stop=True)
            ot = sb.tile([128, 2 * 128], bf16, tag=f"onrm{qd}", name="onrm")
            nc.vector.tensor_mul(
                out=ot, in0=avsb[:, 256 * qd:256 * qd + 256], in1=rdsp)
            for pair in range(2):
                nc.tensor.matmul(
                    out=yps[pair], lhsT=ot[:, 128 * pair:128 * pair + 128],
                    rhs=wo_sp[:, C * qd:C * qd + C],
                    start=(qd == 0), stop=(qd == 5))
        for pair in range(2):
            yt = sb.tile([128, C], f32, tag=f"ysb{pair}", name="ysb")
            copy_alt(yt, yps[pair])
            (nc.sync if pair == 0 else nc.scalar).dma_start(
                out=y_d[128 * pair:128 * pair + 128, :], in_=yt)

    nc.compile()
    return nc


def _get_nc():
    if "nc" not in _CACHE:
        _CACHE["nc"] = _build_nc()
    return _CACHE["nc"]


def make_in_maps(inputs):
    x = np.asarray(inputs["x"], np.float32)
    pos = np.asarray(inputs["pos"], np.float32)
    prep = _host_prep(np.asarray(inputs["Wq"], np.float32),
                      np.asarray(inputs["Wk"], np.float32),
                      np.asarray(inputs["Wv"], np.float32),
                      np.asarray(inputs["Wo"], np.float32),
                      np.asarray(inputs["rope_freqs"], np.float32))
    in_maps = []
    for c in range(NCORES):
        sl = slice(c * NPC, (c + 1) * NPC)
        m = dict(prep)
        xs = x[sl]
        m["x"] = np.ascontiguousarray(
            xs.reshape(2, 128, C).transpose(1, 0, 2).reshape(128, 2 * C)
            .astype(BF))
        m["posT"] = np.ascontiguousarray(pos[sl].T.astype(np.float32))
        in_maps.append(m)
    return in_maps


def kernel(**inputs):
    from concourse.bass_utils import run_bass_kernel_spmd

    in_maps = make_in_maps(inputs)

    nc = _get_nc()
    res = run_bass_kernel_spmd(nc, in_maps, core_ids=list(range(NCORES)))
    out = np.concatenate([res.results[c]["y"] for c in range(NCORES)], axis=0)
    return out.astype(np.float32)


# revision 17
# speedup vs baseline: 1.9043x; 1.0115x over previous
"""PlatonicConv (graph-mode attention) Trainium2 Bass kernel.

Math (per graph of 64 fully-connected nodes, 24 group-heads of dim 16):
  q/k/v = x @ W; RoPE(q, k) from pos; S = q.k^T/4; softmax over dst;
  out = A @ v; y = out @ Wo.  32 graphs -> data-parallel over 8 cores.

v3 design (per core: 4 graphs, 256 nodes), all matmul operands bf16
(full PE rate at any free dim; fp32/f32r are 4x slower below 256 cols):
  * x^T via PE transposes; q/k projected transposed ([feature, node]).
  * RoPE pair-swap folds into spread matrices: rot_spread accumulates
    esp@(q*cos) + espp@(q*sin) in PSUM -- no Wqp/Wkp weights.
  * theta range-reduction: two chained add_range_wrap custom-DVE ops
    (valid for |theta| < 4.5pi; actual max ~10.8), one Sin table.
  * Q/K spreads use a parity-interleaved layout (graph parity g01 at
    16-row offset inside each 32-slot) so ONE block-diagonal matmul
    computes both parities' scores: 48 score matmuls instead of 96.
  * Softmax is max-free (scores O(1) by construction); denominators are
    a 17th row of each AV matmul via an interleaved ones-column in V,
    extracted/broadcast with tiny PE matmuls, reciprocal_approx_fast.
  * AV output stays in spread layout; Wo rows are pre-spread host-side
    (wo_sp); final y lands in natural node order -> dense DRAM writes.
  * DMA queues: sync HWDGE = x/wq/wvil/y0; scalar HWDGE = small consts
    + y1 (keeps the Sin/Exp activations unblocked); gpsimd SWDGE =
    everything else (wk, wo_sp, spread/selector matrices).
"""

import numpy as np
import ml_dtypes

G = 12
H = 2
D = 16
GH = 24          # G * H group-heads
C = 384          # in/emb/out channels
NG = 32          # graphs
NPG = 64         # nodes per graph
N = NG * NPG
NCORES = 8
GPC = NG // NCORES   # graphs per core = 4
NPC = GPC * NPG      # nodes per core = 256
VW = 17              # V block width (16 + ones col)
CAUG = GH * VW       # 408

BF = ml_dtypes.bfloat16

_CACHE = {}


def _host_prep(Wq, Wk, Wv, Wo, rope_freqs):
    f32 = np.float32

    def pack(w):
        # [384, cols] -> [128, 3*cols]: row p = concat_s w[128 s + p]
        cols = w.shape[1]
        return np.ascontiguousarray(
            w.reshape(3, 128, cols).transpose(1, 0, 2).reshape(128, 3 * cols)
            .astype(BF))

    # V interleaved with a ones column per head: block j = [Wv head j | 0]
    Wvil = np.zeros((C, CAUG), f32)
    for j in range(GH):
        Wvil[:, VW * j:VW * j + 16] = Wv[:, 16 * j:16 * j + 16]
    vseed = np.zeros((1, CAUG), f32)
    vseed[0, VW * np.arange(GH) + 16] = 1.0

    # theta pattern [3, 128] for COMPACT layout rows (dup to both halves):
    # row r=16m+d of a 64-block -> head h=m%2, pair w=d//2
    fr = rope_freqs.astype(f32)            # [3, 2, 8]
    fpat = np.zeros((3, 128), f32)
    for r in range(128):
        rr = r % 64
        fpat[:, r] = fr[:, (rr // 16) % 2, (rr % 16) // 2]

    # spread matrices, parity-interleaved: compact row k (16/head) of a
    # 64-row half -> slot row 32s + 16*P + j for graph parity P. Variant
    # 2P   = plain (multiplies q*cos), variant 2P+1 = pair-swap signed
    # (multiplies q*sin). Two stacked copies for odd 64-row slabs.
    espx = np.zeros((128, 4, 128), f32)
    for k in range(64):
        s, j = divmod(k, 16)
        jp = j + 1 if j % 2 == 0 else j - 1
        sg = 1.0 if j % 2 == 0 else -1.0
        for P in range(2):
            espx[k, 2 * P, 32 * s + 16 * P + j] = 1.0
            espx[64 + k, 2 * P, 32 * s + 16 * P + j] = 1.0
            espx[k, 2 * P + 1, 32 * s + 16 * P + jp] = sg
            espx[64 + k, 2 * P + 1, 32 * s + 16 * P + jp] = sg

    # den extract: per quad qd, spread row 32a+16 -> out row 4qd+a of a
    # [24,128] PSUM accumulation (out base partition must be 32-aligned,
    # so each quad writes the full 24 rows, 4 nonzero)
    esel = np.zeros((128, 6 * GH), f32)
    for qd in range(6):
        for a in range(4):
            esel[32 * a + 16, GH * qd + 4 * qd + a] = 1.0

    # den broadcast: rden row gh -> 16 spread rows of its quad block
    bsel = np.zeros((GH, 6 * 128), f32)
    for gh in range(GH):
        qd, a = divmod(gh, 4)
        bsel[gh, 128 * qd + 32 * a:128 * qd + 32 * a + 16] = 1.0

    # Wo with rows pre-spread to the AV output layout (den/pad rows = 0)
    wo_sp = np.zeros((128, 6 * C), f32)
    for gh in range(GH):
        qd, a = divmod(gh, 4)
        wo_sp[32 * a:32 * a + 16, C * qd:C * qd + C] = Wo[16 * gh:16 * gh + 16]

    return dict(
        wq=pack(Wq), wk=pack(Wk), wvil=pack(Wvil),
        wo_sp=wo_sp.astype(BF), vseed=vseed.astype(BF),
        fpat=fpat,
        espx=np.ascontiguousarray(espx.reshape(128, 4 * 128)).astype(BF),
        esel=esel.astype(BF), bsel=bsel.astype(BF),
        onesrow=np.ones((1, 128), BF), ident=np.eye(128, dtype=BF),
    )


def _build_nc():
    import concourse.bacc as bacc
    import concourse.tile as tile
    import concourse.mybir as mybir
    from contextlib import ExitStack

    f32 = mybir.dt.float32
    bf16 = mybir.dt.bfloat16
    AF = mybir.ActivationFunctionType

    nc = bacc.Bacc("TRN2", target_bir_lowering=False)

    x_d = nc.dram_tensor("x", [128, 2 * C], bf16, kind="ExternalInput")
    posT_d = nc.dram_tensor("posT", [3, NPC], f32, kind="ExternalInput")
    wq_d = nc.dram_tensor("wq", [128, 3 * C], bf16, kind="ExternalInput")
    wk_d = nc.dram_tensor("wk", [128, 3 * C], bf16, kind="ExternalInput")
    wvil_d = nc.dram_tensor("wvil", [128, 3 * CAUG], bf16, kind="ExternalInput")
    wo_sp_d = nc.dram_tensor("wo_sp", [128, 6 * C], bf16, kind="ExternalInput")
    vseed_d = nc.dram_tensor("vseed", [1, CAUG], bf16, kind="ExternalInput")
    fpat_d = nc.dram_tensor("fpat", [3, 128], f32, kind="ExternalInput")
    espx_d = nc.dram_tensor("espx", [128, 4 * 128], bf16, kind="ExternalInput")
    esel_d = nc.dram_tensor("esel", [128, 6 * GH], bf16, kind="ExternalInput")
    bsel_d = nc.dram_tensor("bsel", [GH, 6 * 128], bf16, kind="ExternalInput")
    ones_d = nc.dram_tensor("onesrow", [1, 128], bf16, kind="ExternalInput")
    ident_d = nc.dram_tensor("ident", [128, 128], bf16, kind="ExternalInput")
    y_d = nc.dram_tensor("y", [NPC, C], f32, kind="ExternalOutput")

    ctx = ExitStack()
    with tile.TileContext(nc) as tc, ctx:
        consts = ctx.enter_context(tc.tile_pool(name="consts", bufs=1))
        wpool = ctx.enter_context(tc.tile_pool(name="weights", bufs=1))
        sb = ctx.enter_context(tc.tile_pool(name="sbuf", bufs=1))
        # 2+2+4 PSUM banks: gp (proj/theta/vau/den/bcast/y), sp (spread/AV),
        # att (score tiles; bank gh%4 for 4-way concurrent row groups)
        ps_gp = ctx.enter_context(tc.tile_pool(name="ps_gp", bufs=2, space="PSUM"))
        ps_sp = ctx.enter_context(tc.tile_pool(name="ps_sp", bufs=2, space="PSUM"))
        ps_att = ctx.enter_context(tc.tile_pool(name="ps_att", bufs=1, space="PSUM"))

        def gpt(shape, dt=f32):
            return ps_gp.tile(shape, dt, tag="pp", name="pp")

        def spt(shape, dt=f32):
            return ps_sp.tile(shape, dt, tag="sp", name="sp")

        # ---- inputs; x/wq/wvil on sync HWDGE, tiny consts on scalar HWDGE,
        # the rest on gpsimd SWDGE so the scalar engine stays free ----
        xsb = sb.tile([128, 2, C], bf16, tag="x")
        nc.sync.dma_start(out=xsb, in_=x_d.rearrange("p (s e) -> p s e", s=2))
        ident = consts.tile([128, 128], bf16, tag="ident")
        nc.scalar.dma_start(out=ident, in_=ident_d[:])
        posT = consts.tile([3, NPC], f32, tag="posT")
        nc.scalar.dma_start(out=posT, in_=posT_d[:])
        fpat = consts.tile([3, 128], f32, tag="fpat")
        nc.scalar.dma_start(out=fpat, in_=fpat_d[:])
        espx = consts.tile([128, 4, 128], bf16, tag="espx")
        nc.gpsimd.dma_start(out=espx, in_=espx_d.rearrange("p (v e) -> p v e", v=4))
        esel = consts.tile([128, 6 * GH], bf16, tag="esel")
        nc.gpsimd.dma_start(out=esel, in_=esel_d[:])
        bsel = consts.tile([GH, 6 * 128], bf16, tag="bsel")
        nc.gpsimd.dma_start(out=bsel, in_=bsel_d[:])
        vseed = consts.tile([1, CAUG], bf16, tag="vseed")
        nc.gpsimd.dma_start(out=vseed, in_=vseed_d[:])
        onesrow = consts.tile([1, 128], bf16, tag="ones")
        nc.gpsimd.dma_start(out=onesrow, in_=ones_d[:])

        wq = wpool.tile([128, 3, C], bf16, tag="wq")
        nc.sync.dma_start(out=wq, in_=wq_d.rearrange("p (s e) -> p s e", s=3))
        wvil = wpool.tile([128, 3, CAUG], bf16, tag="wvil")
        nc.sync.dma_start(out=wvil, in_=wvil_d.rearrange("p (s e) -> p s e", s=3))
        wk = wpool.tile([128, 3, C], bf16, tag="wk")
        nc.gpsimd.dma_start(out=wk, in_=wk_d.rearrange("p (s e) -> p s e", s=3))
        wo_sp = wpool.tile([128, 6 * C], bf16, tag="wo_sp")
        nc.gpsimd.dma_start(out=wo_sp, in_=wo_sp_d[:])

        # ---- X^T [384, 256] via PE transposes (bf16) ----
        xT = []
        for j in range(3):
            t = sb.tile([128, NPC], bf16, tag=f"xT{j}")
            for i in range(2):
                pst = gpt([128, 128], bf16)
                nc.tensor.transpose(
                    out=pst, in_=xsb[:, i, 128 * j:128 * j + 128], identity=ident)
                nc.vector.tensor_copy(out=t[:, 128 * i:128 * i + 128], in_=pst)
            xT.append(t)

        # ---- theta [128, 256]; range-reduce via 2 chained add_range_wrap
        # (safe for |theta| < 4.5pi); Sin table gives both sin and cos ----
        PI = float(np.pi)
        thps = gpt([128, NPC])
        nc.tensor.matmul(
            out=thps, lhsT=fpat,
            rhs=posT,
            start=True, stop=True)
        cpat = sb.tile([128, NPC], f32, tag="cpat")
        spat = sb.tile([128, NPC], f32, tag="spat")
        for (tgt, shift) in ((spat, 0.0), (cpat, PI / 2)):
            w1 = sb.tile([128, NPC], f32, tag=f"w1{shift}", name="w1")
            nc.vector.add_range_wrap(out=w1, in_=thps, shift=shift,
                                     bound=PI, period=2 * PI)
            w2 = sb.tile([128, NPC], f32, tag=f"w2{shift}", name="w2")
            nc.vector.add_range_wrap(out=w2, in_=w1, shift=0.0,
                                     bound=PI, period=2 * PI)
            nc.scalar.activation(out=tgt, in_=w2, func=AF.Sin)

        # ---- projections (transposed) + RoPE + parity-interleaved spread.
        # qz[t] [128,128]: slot rows 32s+16P+j, cols (pair, src i) -- both
        # parities' q stacked per column.  kz[t] [128, (pr, P, i)]: same rows,
        # block-diag in data (parity P data only in P's 16-row sub-slot). ----
        def proj_m(w, m):
            ps = gpt([128, NPC])
            for k in range(3):
                nc.tensor.matmul(
                    out=ps,
                    lhsT=w[:, k, 128 * m:128 * m + 128],
                    rhs=xT[k],
                    start=(k == 0), stop=(k == 2))
            return ps

        qz, kz = [], []

        def copy_scalar(out, in_):
            # psum->sbuf copy on the scalar engine (idle during proj phase)
            nc.scalar.activation(out=out, in_=in_, func=AF.Copy)

        for m in range(3):
            for (w, out_tiles, isq, tg) in ((wq, qz, True, "q"), (wk, kz, False, "k")):
                qt = proj_m(w, m)
                a = sb.tile([128, 2, 2, 64], bf16, tag=f"ra{tg}{m}")
                b = sb.tile([128, 2, 2, 64], bf16, tag=f"rb{tg}{m}")
                nc.vector.tensor_mul(out=a, in0=qt, in1=cpat)
                nc.vector.tensor_mul(out=b, in0=qt, in1=spat)
                for half in range(2):
                    hs = slice(64 * half, 64 * half + 64)
                    if isq:
                        sp = spt([128, 128])
                        for P in range(2):
                            nc.tensor.matmul(
                                out=sp, lhsT=espx[hs, 2 * P, :],
                                rhs=a[hs, :, P, :],
                                start=(P == 0), stop=False)
                            nc.tensor.matmul(
                                out=sp, lhsT=espx[hs, 2 * P + 1, :],
                                rhs=b[hs, :, P, :],
                                start=False, stop=(P == 1))
                        t = sb.tile([128, 128], bf16, tag=f"sps{tg}{2 * m + half}")
                    else:
                        sp = spt([128, 2, 2, 64])
                        for P in range(2):
                            nc.tensor.matmul(
                                out=sp[:, :, P, :], lhsT=espx[hs, 2 * P, :],
                                rhs=a[hs, :, P, :],
                                start=True, stop=False)
                            nc.tensor.matmul(
                                out=sp[:, :, P, :], lhsT=espx[hs, 2 * P + 1, :],
                                rhs=b[hs, :, P, :],
                                start=False, stop=True)
                        t = sb.tile([128, 2, 2, 64], bf16,
                                    tag=f"sps{tg}{2 * m + half}")
                    copy_scalar(t, sp)
                    out_tiles.append(t)

        # ---- V_aug [256, 408] untransposed (+ ones cols via K=1 matmul) ----
        vau = []
        for i in range(2):
            ps = gpt([128, CAUG])
            for k in range(3):
                nc.tensor.matmul(
                    out=ps,
                    lhsT=xT[k][:, 128 * i:128 * i + 128],
                    rhs=wvil[:, k, :],
                    start=(k == 0), stop=False)
            nc.tensor.matmul(
                out=ps, lhsT=onesrow, rhs=vseed,
                start=False, stop=True)
            t = sb.tile([128, CAUG], bf16, tag=f"vau{i}")
            copy_scalar(t, ps)
            vau.append(t)

        # ---- scores S^T + exp, per graph-pair: ONE block-diag matmul per
        # (gh, pair) covers both parities.  head gh -> bank gh%4 (512-col
        # block), col 64*(gh//4); rows 64*P via the kz data layout. ----
        def scol(gh):
            return 512 * (gh % 4) + 64 * (gh // 4)

        expst = []
        for pair in range(2):
            stp = ps_att.tile([128, 4 * 512], f32, tag="stps")
            for gh in range(GH):
                tilei, slot = divmod(gh, 4)
                lo = 32 * slot
                nc.tensor.matmul(
                    out=stp[:, scol(gh):scol(gh) + 64],
                    lhsT=kz[tilei][lo:lo + 32, pair, :, :],
                    rhs=qz[tilei][lo:lo + 32, 64 * pair:64 * pair + 64],
                    start=True, stop=True,
                    tile_position=(lo, 0))
            et = sb.tile([128, 4 * 512], bf16, tag=f"expst{pair}")
            # 2 chunks per bank so AV quads 0-2 can start before the
            # bank's later columns (tiles 3-5) are scored
            for b in range(4):
                for ch in range(2):
                    co = 512 * b + 192 * ch
                    nc.scalar.activation(
                        out=et[:, co:co + 192],
                        in_=stp[:, co:co + 192],
                        func=AF.Exp, scale=0.25)
            expst.append(et)

        # ---- AV (+den row) per (pair, quad): two [128,64] psum tiles so
        # concurrent graph parities use distinct banks. avsb columns land in
        # natural node order: col = 256*qd + 128*pair + 64*g01 + i64.
        # (no memsets: psum slots hold finite data from the spread phase,
        # and pad rows are zero-masked by rdsp/wo_sp downstream) ----
        avsb = sb.tile([128, 6 * 256], bf16, tag="avsb")
        for pair in range(2):
            for qd in range(6):
                mk = spt if (qd % 2 == 0) else (lambda sh: gpt(sh))
                avt = [mk([128, 64]) for _ in range(2)]
                for a in range(4):
                    gh = 4 * qd + a
                    for g01 in range(2):
                        nc.tensor.matmul(
                            out=avt[g01][32 * a:32 * a + VW, :],
                            lhsT=vau[pair][64 * g01:64 * g01 + 64,
                                           VW * gh:VW * gh + VW],
                            rhs=expst[pair][64 * g01:64 * g01 + 64,
                                            scol(gh):scol(gh) + 64],
                            start=True, stop=True,
                            tile_position=(64 * g01, 32 * a))
                cbase = 256 * qd + 128 * pair
                for g01 in range(2):
                    nc.vector.tensor_copy(
                        out=avsb[:, cbase + 64 * g01:cbase + 64 * g01 + 64],
                        in_=avt[g01])

        # ---- merged tail: den extract (both pairs per quad), reciprocal,
        # broadcast, normalize, project.  den rows (32a+16) -> denps rows
        # 4qd+a (full [24,256] write with 4 nonzero rows, accumulated) ----
        denps = gpt([GH, 2 * 128])
        for qd in range(6):
            nc.tensor.matmul(
                out=denps,
                lhsT=esel[:, GH * qd:GH * qd + GH],
                rhs=avsb[:, 256 * qd:256 * qd + 256],
                start=(qd == 0), stop=(qd == 5))
        rden = sb.tile([GH, 2 * 128], f32, tag="rden")
        nc.vector.reciprocal_approx_fast(out=rden, in_=denps)
        rdenb = sb.tile([GH, 2 * 128], bf16, tag="rdenb")
        nc.vector.tensor_copy(out=rdenb, in_=rden)
        yps = [gpt([128, C]) for _ in range(2)]
        for qd in range(6):
            rdsp = spt([128, 2 * 128])
            nc.tensor.matmul(
                out=rdsp, lhsT=bsel[:, 128 * qd:128 * qd + 128],
                rhs=rdenb, start=True, stop=True)
            ot = sb.tile([128, 2 * 128], bf16, tag=f"onrm{qd}", name="onrm")
            nc.vector.tensor_mul(
                out=ot, in0=avsb[:, 256 * qd:256 * qd + 256], in1=rdsp)
            for pair in range(2):
                nc.tensor.matmul(
                    out=yps[pair], lhsT=ot[:, 128 * pair:128 * pair + 128],
                    rhs=wo_sp[:, C * qd:C * qd + C],
                    start=(qd == 0), stop=(qd == 5))
        for pair in range(2):
            yt = sb.tile([128, C], f32, tag=f"ysb{pair}", name="ysb")
            nc.vector.tensor_copy(out=yt, in_=yps[pair])
            (nc.sync if pair == 0 else nc.scalar).dma_start(
                out=y_d[128 * pair:128 * pair + 128, :], in_=yt)

    nc.compile()
    return nc


def _get_nc():
    if "nc" not in _CACHE:
        _CACHE["nc"] = _build_nc()
    return _CACHE["nc"]


def make_in_maps(inputs):
    x = np.asarray(inputs["x"], np.float32)
    pos = np.asarray(inputs["pos"], np.float32)
    prep = _host_prep(np.asarray(inputs["Wq"], np.float32),
                      np.asarray(inputs["Wk"], np.float32),
                      np.asarray(inputs["Wv"], np.float32),
                      np.asarray(inputs["Wo"], np.float32),
                      np.asarray(inputs["rope_freqs"], np.float32))
    in_maps = []
    for c in range(NCORES):
        sl = slice(c * NPC, (c + 1) * NPC)
        m = dict(prep)
        xs = x[sl]
        m["x"] = np.ascontiguousarray(
            xs.reshape(2, 128, C).transpose(1, 0, 2).reshape(128, 2 * C)
            .astype(BF))
        m["posT"] = np.ascontiguousarray(pos[sl].T.astype(np.float32))
        in_maps.append(m)
    return in_maps


def kernel(**inputs):
    from concourse.bass_utils import run_bass_kernel_spmd

    in_maps = make_in_maps(inputs)

    nc = _get_nc()
    res = run_bass_kernel_spmd(nc, in_maps, core_ids=list(range(NCORES)))
    out = np.concatenate([res.results[c]["y"] for c in range(NCORES)], axis=0)
    return out.astype(np.float32)


# revision 18
# speedup vs baseline: 2.0526x; 1.0779x over previous
"""PlatonicConv (graph-mode attention) Trainium2 Bass kernel.

Math (per graph of 64 fully-connected nodes, 24 group-heads of dim 16):
  q/k/v = x @ W; RoPE(q, k) from pos; S = q.k^T/4; softmax over dst;
  out = A @ v; y = out @ Wo.  32 graphs -> data-parallel over 8 cores.

v3 design (per core: 4 graphs, 256 nodes), all matmul operands bf16
(full PE rate at any free dim; fp32/f32r are 4x slower below 256 cols):
  * x^T via PE transposes; q/k projected transposed ([feature, node]).
  * RoPE pair-swap folds into spread matrices: rot_spread accumulates
    esp@(q*cos) + espp@(q*sin) in PSUM -- no Wqp/Wkp weights.
  * theta range-reduction: two chained add_range_wrap custom-DVE ops
    (valid for |theta| < 4.5pi; actual max ~10.8), one Sin table.
  * Q/K spreads use a parity-interleaved layout (graph parity g01 at
    16-row offset inside each 32-slot) so ONE block-diagonal matmul
    computes both parities' scores: 48 score matmuls instead of 96.
  * Softmax is max-free (scores O(1) by construction); denominators are
    a 17th row of each AV matmul via an interleaved ones-column in V,
    extracted/broadcast with tiny PE matmuls, reciprocal_approx_fast.
  * AV output stays in spread layout; Wo rows are pre-spread host-side
    (wo_sp); final y lands in natural node order -> dense DRAM writes.
  * DMA queues: sync HWDGE = x/wq/wvil/y0; scalar HWDGE = small consts
    + y1 (keeps the Sin/Exp activations unblocked); gpsimd SWDGE =
    everything else (wk, wo_sp, spread/selector matrices).
"""

import numpy as np
import ml_dtypes

G = 12
H = 2
D = 16
GH = 24          # G * H group-heads
C = 384          # in/emb/out channels
NG = 32          # graphs
NPG = 64         # nodes per graph
N = NG * NPG
NCORES = 8
GPC = NG // NCORES   # graphs per core = 4
NPC = GPC * NPG      # nodes per core = 256
VW = 17              # V block width (16 + ones col)
CAUG = GH * VW       # 408

BF = ml_dtypes.bfloat16

_CACHE = {}


def _host_prep(Wq, Wk, Wv, Wo, rope_freqs):
    f32 = np.float32

    def pack(w):
        # [384, cols] -> [128, 3*cols]: row p = concat_s w[128 s + p]
        cols = w.shape[1]
        return np.ascontiguousarray(
            w.reshape(3, 128, cols).transpose(1, 0, 2).reshape(128, 3 * cols)
            .astype(BF))

    # V interleaved with a ones column per head: block j = [Wv head j | 0]
    Wvil = np.zeros((C, CAUG), f32)
    for j in range(GH):
        Wvil[:, VW * j:VW * j + 16] = Wv[:, 16 * j:16 * j + 16]
    vseed = np.zeros((1, CAUG), f32)
    vseed[0, VW * np.arange(GH) + 16] = 1.0

    # theta pattern [3, 128] for COMPACT layout rows (dup to both halves):
    # row r=16m+d of a 64-block -> head h=m%2, pair w=d//2
    fr = rope_freqs.astype(f32)            # [3, 2, 8]
    fpat = np.zeros((3, 128), f32)
    for r in range(128):
        rr = r % 64
        fpat[:, r] = fr[:, (rr // 16) % 2, (rr % 16) // 2]

    # spread matrices, parity-interleaved: compact row k (16/head) of a
    # 64-row half -> slot row 32s + 16*P + j for graph parity P. Variant
    # 2P   = plain (multiplies q*cos), variant 2P+1 = pair-swap signed
    # (multiplies q*sin). Two stacked copies for odd 64-row slabs.
    espx = np.zeros((128, 4, 128), f32)
    for k in range(64):
        s, j = divmod(k, 16)
        jp = j + 1 if j % 2 == 0 else j - 1
        sg = 1.0 if j % 2 == 0 else -1.0
        for P in range(2):
            espx[k, 2 * P, 32 * s + 16 * P + j] = 1.0
            espx[64 + k, 2 * P, 32 * s + 16 * P + j] = 1.0
            espx[k, 2 * P + 1, 32 * s + 16 * P + jp] = sg
            espx[64 + k, 2 * P + 1, 32 * s + 16 * P + jp] = sg

    # den extract: per quad qd, spread row 32a+16 -> out row 4qd+a of a
    # [24,128] PSUM accumulation (out base partition must be 32-aligned,
    # so each quad writes the full 24 rows, 4 nonzero)
    esel = np.zeros((128, 6 * GH), f32)
    for qd in range(6):
        for a in range(4):
            esel[32 * a + 16, GH * qd + 4 * qd + a] = 1.0

    # den broadcast: rden row gh -> 16 spread rows of its quad block
    bsel = np.zeros((GH, 6 * 128), f32)
    for gh in range(GH):
        qd, a = divmod(gh, 4)
        bsel[gh, 128 * qd + 32 * a:128 * qd + 32 * a + 16] = 1.0

    # Wo with rows pre-spread to the AV output layout (den/pad rows = 0)
    wo_sp = np.zeros((128, 6 * C), f32)
    for gh in range(GH):
        qd, a = divmod(gh, 4)
        wo_sp[32 * a:32 * a + 16, C * qd:C * qd + C] = Wo[16 * gh:16 * gh + 16]

    return dict(
        wq=pack(Wq), wk=pack(Wk), wvil=pack(Wvil),
        wo_sp=wo_sp.astype(BF), vseed=vseed.astype(BF),
        fpat=fpat,
        espx=np.ascontiguousarray(espx.reshape(128, 4 * 128)).astype(BF),
        esel=esel.astype(BF), bsel=bsel.astype(BF),
        onesrow=np.ones((1, 128), BF), ident=np.eye(128, dtype=BF),
    )


def _build_nc():
    import concourse.bacc as bacc
    import concourse.tile as tile
    import concourse.mybir as mybir
    from contextlib import ExitStack

    f32 = mybir.dt.float32
    bf16 = mybir.dt.bfloat16
    AF = mybir.ActivationFunctionType

    nc = bacc.Bacc("TRN2", target_bir_lowering=False)

    x_d = nc.dram_tensor("x", [128, 2 * C], bf16, kind="ExternalInput")
    posT_d = nc.dram_tensor("posT", [3, NPC], f32, kind="ExternalInput")
    wq_d = nc.dram_tensor("wq", [128, 3 * C], bf16, kind="ExternalInput")
    wk_d = nc.dram_tensor("wk", [128, 3 * C], bf16, kind="ExternalInput")
    wvil_d = nc.dram_tensor("wvil", [128, 3 * CAUG], bf16, kind="ExternalInput")
    wo_sp_d = nc.dram_tensor("wo_sp", [128, 6 * C], bf16, kind="ExternalInput")
    vseed_d = nc.dram_tensor("vseed", [1, CAUG], bf16, kind="ExternalInput")
    fpat_d = nc.dram_tensor("fpat", [3, 128], f32, kind="ExternalInput")
    espx_d = nc.dram_tensor("espx", [128, 4 * 128], bf16, kind="ExternalInput")
    esel_d = nc.dram_tensor("esel", [128, 6 * GH], bf16, kind="ExternalInput")
    bsel_d = nc.dram_tensor("bsel", [GH, 6 * 128], bf16, kind="ExternalInput")
    ones_d = nc.dram_tensor("onesrow", [1, 128], bf16, kind="ExternalInput")
    ident_d = nc.dram_tensor("ident", [128, 128], bf16, kind="ExternalInput")
    y_d = nc.dram_tensor("y", [NPC, C], f32, kind="ExternalOutput")

    ctx = ExitStack()
    with tile.TileContext(nc) as tc, ctx:
        consts = ctx.enter_context(tc.tile_pool(name="consts", bufs=1))
        wpool = ctx.enter_context(tc.tile_pool(name="weights", bufs=1))
        sb = ctx.enter_context(tc.tile_pool(name="sbuf", bufs=1))
        # 2+2+4 PSUM banks: gp (proj/theta/vau/den/bcast/y), sp (spread/AV),
        # att (score tiles; bank gh%4 for 4-way concurrent row groups)
        ps_gp = ctx.enter_context(tc.tile_pool(name="ps_gp", bufs=2, space="PSUM"))
        ps_sp = ctx.enter_context(tc.tile_pool(name="ps_sp", bufs=2, space="PSUM"))
        ps_att = ctx.enter_context(tc.tile_pool(name="ps_att", bufs=1, space="PSUM"))

        def gpt(shape, dt=f32):
            return ps_gp.tile(shape, dt, tag="pp", name="pp")

        def spt(shape, dt=f32):
            return ps_sp.tile(shape, dt, tag="sp", name="sp")

        # ---- inputs; x/wq/wvil on sync HWDGE, tiny consts on scalar HWDGE,
        # the rest on gpsimd SWDGE so the scalar engine stays free ----
        ident = consts.tile([128, 128], bf16, tag="ident")
        nc.sync.dma_start(out=ident, in_=ident_d[:])
        xsb = sb.tile([128, 2, C], bf16, tag="x")
        nc.sync.dma_start(out=xsb, in_=x_d.rearrange("p (s e) -> p s e", s=2))
        posT = consts.tile([3, NPC], f32, tag="posT")
        nc.sync.dma_start(out=posT, in_=posT_d[:])
        fpat = consts.tile([3, 128], f32, tag="fpat")
        nc.sync.dma_start(out=fpat, in_=fpat_d[:])
        espx = consts.tile([128, 4, 128], bf16, tag="espx")
        nc.gpsimd.dma_start(out=espx, in_=espx_d.rearrange("p (v e) -> p v e", v=4))
        esel = consts.tile([128, 6 * GH], bf16, tag="esel")
        nc.gpsimd.dma_start(out=esel, in_=esel_d[:])
        bsel = consts.tile([GH, 6 * 128], bf16, tag="bsel")
        nc.gpsimd.dma_start(out=bsel, in_=bsel_d[:])
        vseed = consts.tile([1, CAUG], bf16, tag="vseed")
        nc.gpsimd.dma_start(out=vseed, in_=vseed_d[:])
        onesrow = consts.tile([1, 128], bf16, tag="ones")
        nc.gpsimd.dma_start(out=onesrow, in_=ones_d[:])

        wq = wpool.tile([128, 3, C], bf16, tag="wq")
        nc.sync.dma_start(out=wq, in_=wq_d.rearrange("p (s e) -> p s e", s=3))
        wvil = wpool.tile([128, 3, CAUG], bf16, tag="wvil")
        nc.sync.dma_start(out=wvil, in_=wvil_d.rearrange("p (s e) -> p s e", s=3))
        wk = wpool.tile([128, 3, C], bf16, tag="wk")
        nc.gpsimd.dma_start(out=wk, in_=wk_d.rearrange("p (s e) -> p s e", s=3))
        wo_sp = wpool.tile([128, 6 * C], bf16, tag="wo_sp")
        nc.gpsimd.dma_start(out=wo_sp, in_=wo_sp_d[:])

        # ---- X^T [384, 256] via PE transposes (bf16) ----
        xT = []
        for j in range(3):
            t = sb.tile([128, NPC], bf16, tag=f"xT{j}")
            for i in range(2):
                pst = gpt([128, 128], bf16)
                nc.tensor.transpose(
                    out=pst, in_=xsb[:, i, 128 * j:128 * j + 128], identity=ident)
                nc.vector.tensor_copy(out=t[:, 128 * i:128 * i + 128], in_=pst)
            xT.append(t)

        # ---- theta [128, 256]; range-reduce via 2 chained add_range_wrap
        # (safe for |theta| < 4.5pi); Sin table gives both sin and cos ----
        PI = float(np.pi)
        thps = gpt([128, NPC])
        nc.tensor.matmul(
            out=thps, lhsT=fpat,
            rhs=posT,
            start=True, stop=True)
        cpat = sb.tile([128, NPC], f32, tag="cpat")
        spat = sb.tile([128, NPC], f32, tag="spat")
        for (tgt, shift) in ((spat, 0.0), (cpat, PI / 2)):
            w1 = sb.tile([128, NPC], f32, tag=f"w1{shift}", name="w1")
            nc.vector.add_range_wrap(out=w1, in_=thps, shift=shift,
                                     bound=PI, period=2 * PI)
            w2 = sb.tile([128, NPC], f32, tag=f"w2{shift}", name="w2")
            nc.vector.add_range_wrap(out=w2, in_=w1, shift=0.0,
                                     bound=PI, period=2 * PI)
            nc.scalar.activation(out=tgt, in_=w2, func=AF.Sin)

        # ---- projections (transposed) + RoPE + parity-interleaved spread.
        # qz[t] [128,128]: slot rows 32s+16P+j, cols (pair, src i) -- both
        # parities' q stacked per column.  kz[t] [128, (pr, P, i)]: same rows,
        # block-diag in data (parity P data only in P's 16-row sub-slot). ----
        def proj_m(w, m):
            ps = gpt([128, NPC])
            for k in range(3):
                nc.tensor.matmul(
                    out=ps,
                    lhsT=w[:, k, 128 * m:128 * m + 128],
                    rhs=xT[k],
                    start=(k == 0), stop=(k == 2))
            return ps

        qz, kz = [], []

        def copy_scalar(out, in_):
            # psum->sbuf copy on the scalar engine (idle during proj phase)
            nc.scalar.activation(out=out, in_=in_, func=AF.Copy)

        for m in range(3):
            for (w, out_tiles, isq, tg) in ((wq, qz, True, "q"), (wk, kz, False, "k")):
                qt = proj_m(w, m)
                a = sb.tile([128, 2, 2, 64], bf16, tag=f"ra{tg}{m}")
                b = sb.tile([128, 2, 2, 64], bf16, tag=f"rb{tg}{m}")
                nc.vector.tensor_mul(out=a, in0=qt, in1=cpat)
                nc.vector.tensor_mul(out=b, in0=qt, in1=spat)
                for half in range(2):
                    hs = slice(64 * half, 64 * half + 64)
                    if isq:
                        sp = spt([128, 128])
                        for P in range(2):
                            nc.tensor.matmul(
                                out=sp, lhsT=espx[hs, 2 * P, :],
                                rhs=a[hs, :, P, :],
                                start=(P == 0), stop=False)
                            nc.tensor.matmul(
                                out=sp, lhsT=espx[hs, 2 * P + 1, :],
                                rhs=b[hs, :, P, :],
                                start=False, stop=(P == 1))
                        t = sb.tile([128, 128], bf16, tag=f"sps{tg}{2 * m + half}")
                    else:
                        sp = spt([128, 2, 2, 64])
                        for P in range(2):
                            nc.tensor.matmul(
                                out=sp[:, :, P, :], lhsT=espx[hs, 2 * P, :],
                                rhs=a[hs, :, P, :],
                                start=True, stop=False)
                            nc.tensor.matmul(
                                out=sp[:, :, P, :], lhsT=espx[hs, 2 * P + 1, :],
                                rhs=b[hs, :, P, :],
                                start=False, stop=True)
                        t = sb.tile([128, 2, 2, 64], bf16,
                                    tag=f"sps{tg}{2 * m + half}")
                    nc.vector.tensor_copy(out=t, in_=sp)
                    out_tiles.append(t)

        # ---- V_aug [256, 408] untransposed (+ ones cols via K=1 matmul) ----
        vau = []
        for i in range(2):
            ps = gpt([128, CAUG])
            for k in range(3):
                nc.tensor.matmul(
                    out=ps,
                    lhsT=xT[k][:, 128 * i:128 * i + 128],
                    rhs=wvil[:, k, :],
                    start=(k == 0), stop=False)
            nc.tensor.matmul(
                out=ps, lhsT=onesrow, rhs=vseed,
                start=False, stop=True)
            t = sb.tile([128, CAUG], bf16, tag=f"vau{i}")
            copy_scalar(t, ps)
            vau.append(t)

        # ---- scores S^T + exp, per graph-pair: ONE block-diag matmul per
        # (gh, pair) covers both parities.  head gh -> bank gh%4 (512-col
        # block), col 64*(gh//4); rows 64*P via the kz data layout. ----
        def scol(gh):
            return 512 * (gh % 4) + 64 * (gh // 4)

        expst = []
        for pair in range(2):
            stp = ps_att.tile([128, 4 * 512], f32, tag="stps")
            et = sb.tile([128, 4 * 512], bf16, tag=f"expst{pair}")
            # 2 chunks per bank; chunk 0 (tiles 0-2 cols) is emitted as
            # soon as its scores are done so AV quads 0-2 start early
            for gh in range(GH):
                tilei, slot = divmod(gh, 4)
                lo = 32 * slot
                nc.tensor.matmul(
                    out=stp[:, scol(gh):scol(gh) + 64],
                    lhsT=kz[tilei][lo:lo + 32, pair, :, :],
                    rhs=qz[tilei][lo:lo + 32, 64 * pair:64 * pair + 64],
                    start=True, stop=True,
                    tile_position=(lo, 0))
                if gh == 11:
                    for b in range(4):
                        co = 512 * b
                        nc.scalar.activation(
                            out=et[:, co:co + 192], in_=stp[:, co:co + 192],
                            func=AF.Exp, scale=0.25)
            for b in range(4):
                co = 512 * b + 192
                nc.scalar.activation(
                    out=et[:, co:co + 192], in_=stp[:, co:co + 192],
                    func=AF.Exp, scale=0.25)
            expst.append(et)

        # ---- AV (+den row) per (pair, quad): two [128,64] psum tiles so
        # concurrent graph parities use distinct banks. avsb columns land in
        # natural node order: col = 256*qd + 128*pair + 64*g01 + i64.
        # (no memsets: psum slots hold finite data from the spread phase,
        # and pad rows are zero-masked by rdsp/wo_sp downstream) ----
        avsb = sb.tile([128, 6 * 256], bf16, tag="avsb")
        for pair in range(2):
            for qd in range(6):
                mk = spt if (qd % 2 == 0) else (lambda sh: gpt(sh))
                avt = [mk([128, 64]) for _ in range(2)]
                for a in range(4):
                    gh = 4 * qd + a
                    for g01 in range(2):
                        nc.tensor.matmul(
                            out=avt[g01][32 * a:32 * a + VW, :],
                            lhsT=vau[pair][64 * g01:64 * g01 + 64,
                                           VW * gh:VW * gh + VW],
                            rhs=expst[pair][64 * g01:64 * g01 + 64,
                                            scol(gh):scol(gh) + 64],
                            start=True, stop=True,
                            tile_position=(64 * g01, 32 * a))
                cbase = 256 * qd + 128 * pair
                for g01 in range(2):
                    nc.vector.tensor_copy(
                        out=avsb[:, cbase + 64 * g01:cbase + 64 * g01 + 64],
                        in_=avt[g01])

        # ---- merged tail: den extract (both pairs per quad), reciprocal,
        # broadcast, normalize, project.  den rows (32a+16) -> denps rows
        # 4qd+a (full [24,256] write with 4 nonzero rows, accumulated) ----
        denps = gpt([GH, 2 * 128])
        for qd in range(6):
            nc.tensor.matmul(
                out=denps,
                lhsT=esel[:, GH * qd:GH * qd + GH],
                rhs=avsb[:, 256 * qd:256 * qd + 256],
                start=(qd == 0), stop=(qd == 5))
        rden = sb.tile([GH, 2 * 128], f32, tag="rden")
        nc.vector.reciprocal_approx_fast(out=rden, in_=denps)
        rdenb = sb.tile([GH, 2 * 128], bf16, tag="rdenb")
        nc.vector.tensor_copy(out=rdenb, in_=rden)
        yps = [gpt([128, C]) for _ in range(2)]
        for qd in range(6):
            rdsp = spt([128, 2 * 128])
            nc.tensor.matmul(
                out=rdsp, lhsT=bsel[:, 128 * qd:128 * qd + 128],
                rhs=rdenb, start=True, stop=True)
            ot = sb.tile([128, 2 * 128], bf16, tag=f"onrm{qd}", name="onrm")
            nc.vector.tensor_mul(
                out=ot, in0=avsb[:, 256 * qd:256 * qd + 256], in1=rdsp)
            for pair in range(2):
                nc.tensor.matmul(
                    out=yps[pair], lhsT=ot[:, 128 * pair:128 * pair + 128],
                    rhs=wo_sp[:, C * qd:C * qd + C],
                    start=(qd == 0), stop=(qd == 5))
        for pair in range(2):
            yt = sb.tile([128, C], f32, tag=f"ysb{pair}", name="ysb")
            nc.vector.tensor_copy(out=yt, in_=yps[pair])
            (nc.sync if pair == 0 else nc.scalar).dma_start(
                out=y_d[128 * pair:128 * pair + 128, :], in_=yt)

    nc.compile()
    return nc


def _get_nc():
    if "nc" not in _CACHE:
        _CACHE["nc"] = _build_nc()
    return _CACHE["nc"]


def make_in_maps(inputs):
    x = np.asarray(inputs["x"], np.float32)
    pos = np.asarray(inputs["pos"], np.float32)
    prep = _host_prep(np.asarray(inputs["Wq"], np.float32),
                      np.asarray(inputs["Wk"], np.float32),
                      np.asarray(inputs["Wv"], np.float32),
                      np.asarray(inputs["Wo"], np.float32),
                      np.asarray(inputs["rope_freqs"], np.float32))
    in_maps = []
    for c in range(NCORES):
        sl = slice(c * NPC, (c + 1) * NPC)
        m = dict(prep)
        xs = x[sl]
        m["x"] = np.ascontiguousarray(
            xs.reshape(2, 128, C).transpose(1, 0, 2).reshape(128, 2 * C)
            .astype(BF))
        m["posT"] = np.ascontiguousarray(pos[sl].T.astype(np.float32))
        in_maps.append(m)
    return in_maps


def kernel(**inputs):
    from concourse.bass_utils import run_bass_kernel_spmd

    in_maps = make_in_maps(inputs)

    nc = _get_nc()
    res = run_bass_kernel_spmd(nc, in_maps, core_ids=list(range(NCORES)))
    out = np.concatenate([res.results[c]["y"] for c in range(NCORES)], axis=0)
    return out.astype(np.float32)
